# revision 1
# baseline (speedup 1.0000x reference)
"""Trainium2 Bass kernel for nn_GAT_GCN (gnn_message_passing), 8 NeuronCores.

Strategy:
 - Shard destination nodes across the 8 cores, aligned to graph boundaries
   (16 graphs/core), edges pre-sorted by dst on the host.
 - Segment softmax/scatter-add are done as matmuls against host-built 0/1
   selection blocks (S01); GCN's symmetric norm is folded into the S values.
 - Per-edge gathers use dma_gather (int16 idxs, 256B-multiple fp16 rows).
 - GAT1/GCN1 gather raw x (transposed) and project on the fly; GAT2 gathers a
   pre-projected [h2 | asrc2] table that is exchanged with one AllGather.
 - Softmax skips the segment-max shift (mathematically identical, logits tiny).
"""
import sys
sys.path.insert(0, '/opt/trn_rl_repo')
import numpy as np

N, E, G, F, H = 16384, 131072, 128, 78, 10
NCORE, GPC = 8, 16          # cores, graphs per core
HF = H * F                  # 780
WAUG1 = HF + F + H          # 868 = h1(780) | hgcn(78) | asrc(10)
T2W = HF + H                # 790 = h2 | asrc2
W2AUG = HF + 2 * H          # 800 = h2 | asrc2 | adst2
TROW = 896                  # T2/x1f table row, fp16 (1792B, %256)
XROW = 128                  # x / x2 table row, fp16 (256B)


def _wrap16(v):
    """dma_gather idx layout: [128, len/16] int16, idx i at (i%16, i//16),
    replicated across the 8 Q7 core groups."""
    v = np.asarray(v, np.int16)
    assert len(v) % 16 == 0
    m = v.reshape(-1, 16).T            # [16, S]
    return np.tile(m, (8, 1)).copy()   # [128, S]


def _f16(a):
    return np.ascontiguousarray(np.asarray(a, np.float32)).astype(np.float16)


def prep(x, edge_index, batch, target, Wg1, as1, ad1, bg1, Wg2, as2, ad2, bg2,
         Wgcn, bgcn, Wfg1, bfg1, Wfg2, bfg2, wconv, bconv, Wxt, bxt,
         W1, b1, W2, b2, Wo, bo):
    x = np.asarray(x, np.float32)
    ei = np.asarray(edge_index, np.int64)
    batch = np.asarray(batch, np.int64)
    target = np.asarray(target, np.float32)

    loops = np.arange(N, dtype=np.int64)
    src = np.concatenate([ei[0], loops])
    dst = np.concatenate([ei[1], loops])

    # graph-aligned core boundaries
    counts = np.bincount(batch, minlength=G)
    node_off = np.concatenate([[0], np.cumsum(counts)])
    n_lo = node_off[np.arange(NCORE) * GPC]
    n_hi = node_off[(np.arange(NCORE) + 1) * GPC]

    # degrees / gcn norm (over full edge list incl self loops)
    deg = np.bincount(dst, minlength=N).astype(np.float64)
    dinv = 1.0 / np.sqrt(deg)
    norm = (dinv[src] * dinv[dst]).astype(np.float32)

    order = np.argsort(dst, kind='stable')
    srcs, dsts, norms = src[order], dst[order], norm[order]

    Lmax = int((n_hi - n_lo).max())
    NBLK = (Lmax + 127) // 128
    NPC = NBLK * 128
    assert NCORE * NPC < 32768

    owner = np.searchsorted(n_hi, src, side='right')   # owner core of each node id? (by src value)
    node_owner = np.searchsorted(n_hi - 1, np.arange(N), side='left')
    node_owner = np.minimum(node_owner, NCORE - 1)
    # padded global id used for allgathered tables
    pad_gid = node_owner * NPC + (np.arange(N) - n_lo[node_owner])

    # per (core, block) edge spans -> uniform TPB
    spans = []
    TPB = 1
    for c in range(NCORE):
        e0 = np.searchsorted(dsts, n_lo[c])
        e1 = np.searchsorted(dsts, n_hi[c])
        bl = []
        for b in range(NBLK):
            lo = np.searchsorted(dsts, n_lo[c] + 128 * b)
            hi = np.searchsorted(dsts, min(n_lo[c] + 128 * (b + 1), n_hi[c]))
            if n_lo[c] + 128 * b >= n_hi[c]:
                lo = hi = e1
            bl.append((lo, hi))
            TPB = max(TPB, (hi - lo + 127) // 128)
        spans.append(bl)
    ET = NBLK * TPB
    ECAP = ET * 128

    PW = int(np.ceil(counts.max() / 16) * 16)   # pool slot width per graph

    cores = []
    for c in range(NCORE):
        esrc = np.zeros(ECAP, np.int64)           # raw src id per edge slot
        s01 = np.zeros((ET, 128, 128), np.float16)
        snrm = np.zeros((ET, 128, 128), np.float16)
        s01t = np.zeros((NBLK, 128, TPB * 128), np.float16)
        for b in range(NBLK):
            lo, hi = spans[c][b]
            ne = hi - lo
            if ne == 0:
                continue
            sl = slice(b * TPB * 128, b * TPB * 128 + ne)
            esrc[sl] = srcs[lo:hi]
            ld = (dsts[lo:hi] - n_lo[c] - 128 * b).astype(np.int64)  # 0..127
            j = np.arange(ne)
            t_loc = j // 128
            e_loc = j % 128
            s01[b * TPB + t_loc, e_loc, ld] = 1.0
            snrm[b * TPB + t_loc, e_loc, ld] = norms[lo:hi].astype(np.float16)
            s01t[b, ld, j] = 1.0
        # padded dst columns (no incoming edges) get one dummy S entry so the
        # softmax denominator stays finite (their rows are garbage, never read)
        Lc = int(n_hi[c] - n_lo[c])
        for b in range(NBLK):
            first_pad = max(0, min(128, Lc - 128 * b))
            if first_pad < 128:
                s01[b * TPB, 0, first_pad:] = 1.0

        # pooling indices (local node ids into x1f/x2f tables)
        pool_idx = np.zeros(GPC * PW, np.int64)
        for g in range(GPC):
            gg = c * GPC + g
            a, bnd = node_off[gg] - n_lo[c], node_off[gg + 1] - n_lo[c]
            cnt = bnd - a
            pool_idx[g * PW:g * PW + cnt] = np.arange(a, bnd)
            pool_idx[g * PW + cnt:(g + 1) * PW] = a      # pad = first node of graph
        mmean = np.zeros((NBLK, 128, GPC), np.float16)
        for g in range(GPC):
            gg = c * GPC + g
            a, bnd = node_off[gg] - n_lo[c], node_off[gg + 1] - n_lo[c]
            ids = np.arange(a, bnd)
            mmean[ids // 128, ids % 128, g] = np.float16(1.0 / (bnd - a))

        # conv im2col: [32, GPC, 608]
        t_win = np.zeros((32, GPC, 608), np.float16)
        tg = target[c * GPC:(c + 1) * GPC, 0, :]          # [GPC, 625]
        for k in range(32):
            t_win[k, :, :594] = tg[:, k:k + 594].astype(np.float16)

        L = int(n_hi[c] - n_lo[c])
        xT = np.zeros((128, NPC), np.float16)
        xT[:F, :L] = x[n_lo[c]:n_hi[c], :].T.astype(np.float16)

        s_comb = np.zeros((NBLK, 128, TPB * 256), np.float16)
        for b in range(NBLK):
            for k in range(TPB):
                s_comb[b, :, k * 256:k * 256 + 128] = s01[b * TPB + k]
                s_comb[b, :, k * 256 + 128:k * 256 + 256] = snrm[b * TPB + k]
        cores.append(dict(
            ix_x=_wrap16(esrc),                     # for x-gather (raw ids)
            ix_t2=_wrap16(pad_gid[esrc]),           # for T2/x2 gathers (padded ids)
            ix_pool=_wrap16(pool_idx),
            s01=s01, snrm=snrm, s01t=s01t, s_comb=s_comb,
            mmean=mmean, t_win=t_win, xT_loc=xT,
            bconv_rep=np.full((GPC, 1), float(bconv[0]), np.float32),
        ))

    x16 = np.zeros((N, XROW), np.float16)
    x16[:, :F] = x.astype(np.float16)

    Wg1cat = np.zeros((128, WAUG1), np.float16)
    Wg1cat[:F, :HF] = _f16(Wg1)
    Wg1cat[:F, HF:HF + F] = _f16(Wgcn)
    W2chunks = np.zeros((7, 128, W2AUG), np.float16)
    for k in range(7):
        r0, r1 = 128 * k, min(128 * (k + 1), HF)
        W2chunks[k, :r1 - r0, :HF] = _f16(Wg2[r0:r1, :])
    bg1ch = np.zeros((7, 128, 1), np.float16)
    bg1f = _f16(bg1).reshape(-1)
    for k in range(7):
        r0, r1 = 128 * k, min(128 * (k + 1), HF)
        bg1ch[k, :r1 - r0, 0] = bg1f[r0:r1]

    def pack_rows(Wm, splits, ncol):
        out = np.zeros((len(splits), 128, ncol), np.float16)
        for i, (r0, r1) in enumerate(splits):
            out[i, :r1 - r0, :] = _f16(Wm[r0:r1, :])
        return out

    sp7 = [(128 * i, min(128 * (i + 1), HF)) for i in range(7)]
    wfg1p = np.concatenate([pack_rows(Wfg1[:HF], sp7, 128),
                            pack_rows(Wfg1[HF:], sp7, 128)], axis=0)  # [14,128,128]
    wfg2p = pack_rows(Wfg2, [(0, F), (F, 2 * F)], 128)                # [2,128,128]
    wxtp = pack_rows(Wxt, [(128 * i, min(128 * (i + 1), 594)) for i in range(5)], 256)
    w1p = pack_rows(W1, [(128 * i, 128 * (i + 1)) for i in range(4)], 512)
    w2p = pack_rows(W2, [(128 * i, 128 * (i + 1)) for i in range(4)], 256)
    wop = pack_rows(Wo, [(0, 128), (128, 256)], 1)

    wgcn_s = np.zeros((128, F), np.float16)
    wgcn_s[:F] = _f16(Wgcn)
    bgcn_col = np.zeros((128, 1), np.float32)
    bgcn_col[:F, 0] = np.asarray(bgcn, np.float32)

    shared = dict(
        x16=x16, Wg1cat=Wg1cat, W2chunks=W2chunks, bg1ch=bg1ch,
        as1f=_f16(as1).reshape(1, HF), ad1f=_f16(ad1).reshape(1, HF),
        as2f=_f16(as2).reshape(1, HF), ad2f=_f16(ad2).reshape(1, HF),
        wgcn_s=wgcn_s, bgcn_col=bgcn_col,
        bgcn_row=np.asarray(bgcn, np.float32).reshape(1, F),
        bg2row=np.asarray(bg2, np.float32).reshape(1, HF),
        wfg1p=wfg1p, bfg1=np.asarray(bfg1, np.float32).reshape(1, 128),
        wfg2p=wfg2p, bfg2=np.asarray(bfg2, np.float32).reshape(1, 128),
        wxtp=wxtp, bxt=np.asarray(bxt, np.float32).reshape(1, 256),
        w1p=w1p, b1=np.asarray(b1, np.float32).reshape(1, 512),
        w2p=w2p, b2=np.asarray(b2, np.float32).reshape(1, 256),
        wop=wop, bo_rep=np.full((GPC, 1), float(np.asarray(bo).reshape(-1)[0]), np.float32),
        w_col=np.zeros((32, 1), np.float16),
        w_sel=np.zeros((32, GPC, GPC), np.float16),
    )
    shared['w_col'][:, 0] = _f16(np.asarray(wconv).reshape(-1))
    for g in range(GPC):
        shared['w_sel'][:, g, g] = shared['w_col'][:, 0]

    meta = dict(NBLK=NBLK, NPC=NPC, TPB=TPB, ET=ET, ECAP=ECAP, PW=PW,
                n_lo=n_lo, n_hi=n_hi)
    return meta, shared, cores


# ---------------------------------------------------------------- numpy sim

def unwrap16(m):
    """inverse of _wrap16: [128, S] -> [S*16] (first 16-partition group)."""
    return np.asarray(m[:16, :].T.reshape(-1), np.int64)

import concourse.bass as bass
import concourse.bacc as bacc
import concourse.mybir as mybir
from concourse import library_config
from concourse.tile import TileContext
from concourse.masks import make_identity
from concourse.bass_utils import run_bass_kernel_spmd

F16 = mybir.dt.float16
F32 = mybir.dt.float32
I16 = mybir.dt.int16
AX = mybir.AxisListType.X
ALU = mybir.AluOpType
AF = mybir.ActivationFunctionType



def build(meta):
    NBLK, NPC, TPB, ET, ECAP, PW = (meta[k] for k in
                                    ['NBLK', 'NPC', 'TPB', 'ET', 'ECAP', 'PW'])
    EPB = TPB * 128                       # edges per block
    nc = bacc.Bacc()

    dp = lambda n, s, d: nc.declare_dram_parameter(n, list(s), d, isOutput=False)
    # per-core inputs
    x16 = dp('x16', [N, XROW], F16)
    xT_loc = dp('xT_loc', [128, NPC], F16)
    ix_x = dp('ix_x', [128, ECAP // 16], I16)
    ix_t2 = dp('ix_t2', [128, ECAP // 16], I16)
    ix_pool = dp('ix_pool', [128, GPC * PW // 16], I16)
    scomb_d = dp('s_comb', [NBLK, 128, TPB * 256], F16)
    s01t_d = dp('s01t', [NBLK, 128, EPB], F16)
    mmean_d = dp('mmean', [NBLK, 128, GPC], F16)
    twin_d = dp('t_win', [32, GPC, 608], F16)
    bconv_rep = dp('bconv_rep', [GPC, 1], F32)
    # shared weights
    wg1cat = dp('Wg1cat', [128, WAUG1], F16)
    w2ch = dp('W2chunks', [7, 128, W2AUG], F16)
    bg1ch = dp('bg1ch', [7, 128, 1], F16)
    as1f, ad1f = dp('as1f', [1, HF], F16), dp('ad1f', [1, HF], F16)
    as2f, ad2f = dp('as2f', [1, HF], F16), dp('ad2f', [1, HF], F16)
    wgcn = dp('wgcn_s', [128, F], F16)
    bgcn_col = dp('bgcn_col', [128, 1], F32)
    bgcn_row = dp('bgcn_row', [1, F], F32)
    bg2row = dp('bg2row', [1, HF], F32)
    wfg1p = dp('wfg1p', [14, 128, 128], F16)
    bfg1 = dp('bfg1', [1, 128], F32)
    wfg2p = dp('wfg2p', [2, 128, 128], F16)
    bfg2 = dp('bfg2', [1, 128], F32)
    wxtp = dp('wxtp', [5, 128, 256], F16)
    bxt = dp('bxt', [1, 256], F32)
    w1p = dp('w1p', [4, 128, 512], F16)
    b1 = dp('b1', [1, 512], F32)
    w2p = dp('w2p', [4, 128, 256], F16)
    b2 = dp('b2', [1, 256], F32)
    wop = dp('wop', [2, 128, 1], F16)
    bo_rep = dp('bo_rep', [GPC, 1], F32)
    wcol_d = dp('w_col', [32, 1], F16)
    wsel_d = dp('w_sel', [32, GPC, GPC], F16)

    out_d = nc.declare_dram_parameter('out', [GPC, 1], F32, isOutput=True)

    # internal DRAM
    CROW = 1024
    comb_shard = nc.dram_tensor('comb_shard', [NPC, CROW], F16)
    comb_full = nc.dram_tensor('comb_full', [8 * NPC, CROW], F16, addr_space="Shared")
    x1f_dram = nc.dram_tensor('x1f_dram', [NPC, TROW], F16)
    x2f_dram = nc.dram_tensor('x2f_dram', [NPC, XROW], F16)

    RG = [list(range(8))]

    with TileContext(nc) as tc:
        nc.gpsimd.load_library(library_config.mlp)

        with tc.tile_pool(name="persist", bufs=1) as pp:
            # ---------------- persistent tiles + loads
            w1aug_s = pp.tile([128, WAUG1], F16, tag="w1aug")
            nc.sync.dma_start(out=w1aug_s[:], in_=wg1cat[:])
            w2aug_s = pp.tile([128, 7, W2AUG], F16, tag="w2aug")
            for k in range(7):
                nc.sync.dma_start(out=w2aug_s[:, k, :], in_=w2ch[k])
            bg1_s = pp.tile([128, 7, 1], F16, tag="bg1")
            for k in range(7):
                nc.sync.dma_start(out=bg1_s[:, k, :], in_=bg1ch[k])
            a_s = pp.tile([128, 4, HF], F16, tag="aflat")
            for i, t in enumerate([as1f, ad1f, as2f, ad2f]):
                nc.sync.dma_start(out=a_s[:, i, :], in_=t[:].to_broadcast([128, HF]))
            xT_s = pp.tile([128, NPC], F16, tag="xT")
            nc.sync.dma_start(out=xT_s[:], in_=xT_loc[:])
            ixx_s = pp.tile([128, ECAP // 16], I16, tag="ixx")
            nc.sync.dma_start(out=ixx_s[:], in_=ix_x[:])
            ixt2_s = pp.tile([128, ECAP // 16], I16, tag="ixt2")
            nc.sync.dma_start(out=ixt2_s[:], in_=ix_t2[:])
            ixp_s = pp.tile([128, GPC * PW // 16], I16, tag="ixp")
            nc.sync.dma_start(out=ixp_s[:], in_=ix_pool[:])
            wgcn_s = pp.tile([128, F], F16, tag="wgcn")
            nc.sync.dma_start(out=wgcn_s[:], in_=wgcn[:])
            bgcnc_s = pp.tile([128, 1], F32, tag="bgcnc")
            nc.sync.dma_start(out=bgcnc_s[:], in_=bgcn_col[:])
            bgcnr_s = pp.tile([128, F], F32, tag="bgcnr")
            nc.sync.dma_start(out=bgcnr_s[:], in_=bgcn_row[:].to_broadcast([128, F]))
            bg2_s = pp.tile([128, HF], F32, tag="bg2")
            nc.sync.dma_start(out=bg2_s[:], in_=bg2row[:].to_broadcast([128, HF]))
            mmean_s = pp.tile([128, NBLK, GPC], F16, tag="mmean")
            for b in range(NBLK):
                nc.sync.dma_start(out=mmean_s[:, b, :], in_=mmean_d[b])
            wcol_s = pp.tile([32, 1], F16, tag="wcol")
            nc.sync.dma_start(out=wcol_s[:], in_=wcol_d[:])
            wsel_s = pp.tile([32, GPC, GPC], F16, tag="wsel")
            nc.sync.dma_start(out=wsel_s[:], in_=wsel_d[:])
            bconv_s = pp.tile([GPC, 1], F32, tag="bconv")
            nc.sync.dma_start(out=bconv_s[:], in_=bconv_rep[:])
            wfg1_s = pp.tile([128, 14, 128], F16, tag="wfg1")
            for i in range(14):
                nc.sync.dma_start(out=wfg1_s[:, i, :], in_=wfg1p[i])
            wfg2_s = pp.tile([128, 2, 128], F16, tag="wfg2")
            for i in range(2):
                nc.sync.dma_start(out=wfg2_s[:, i, :], in_=wfg2p[i])
            wxt_s = pp.tile([128, 5, 256], F16, tag="wxt")
            for i in range(5):
                nc.sync.dma_start(out=wxt_s[:, i, :], in_=wxtp[i])
            w1_s = pp.tile([128, 4, 512], F16, tag="w1")
            for i in range(4):
                nc.sync.dma_start(out=w1_s[:, i, :], in_=w1p[i])
            w2_s = pp.tile([128, 4, 256], F16, tag="w2")
            for i in range(4):
                nc.sync.dma_start(out=w2_s[:, i, :], in_=w2p[i])
            wo_s = pp.tile([128, 2, 1], F16, tag="wo")
            for i in range(2):
                nc.sync.dma_start(out=wo_s[:, i, :], in_=wop[i])
            bias_s = {}
            for nm, t, w in [('bfg1', bfg1, 128), ('bfg2', bfg2, 128),
                             ('bxt', bxt, 256), ('b1', b1, 512), ('b2', b2, 256)]:
                bias_s[nm] = pp.tile([GPC, w], F32, tag="bias_" + nm, name="bias_" + nm)
                nc.sync.dma_start(out=bias_s[nm][:], in_=t[:].to_broadcast([GPC, w]))
            bo_s = pp.tile([GPC, 1], F32, tag="bo")
            nc.sync.dma_start(out=bo_s[:], in_=bo_rep[:])

            ident_s = pp.tile([128, 128], F16, tag="ident")
            make_identity(nc, ident_s[:])
            ones_s = pp.tile([1, 128], F16, tag="ones")
            nc.vector.memset(ones_s[:], 1.0)

            # work state
            bd1_s = pp.tile([128, H], F16, tag="bd1")
            adst1_s = pp.tile([128, NBLK, H], F16, tag="adst1")
            adst2_s = pp.tile([128, NBLK, H], F16, tag="adst2")
            x1loc_s = pp.tile([128, NBLK, HF], F16, tag="x1loc")
            agg1_s = pp.tile([128, NBLK, F], F16, tag="agg1")
            c2_s = pp.tile([1, W2AUG], F16, tag="c2")
            exA = pp.tile([128, H + 1], F16, tag="exA")
            exB = pp.tile([128, H + 1], F16, tag="exB")
            nc.vector.memset(exA[:], 1.0)
            nc.vector.memset(exB[:], 1.0)
            t2stage = pp.tile([128, TROW], F16, tag="t2stage")
            nc.vector.memset(t2stage[:], 0.0)
            xstage = pp.tile([128, XROW], F16, tag="xstage")
            nc.vector.memset(xstage[:], 0.0)

            # ---------------- B matrices (device)
            with tc.tile_pool(name="bprep", bufs=2) as bp, \
                 tc.tile_pool(name="bprep_ps", bufs=2, space="PSUM") as bps:
                for (src_w, col0) in [(0, HF), (1, HF + H)]:   # as2 -> B_s2, ad2 -> B_d2
                    pass
                # B_s1 / B_d1 from Wg1 (rows of w1aug_s)
                for i, dst in enumerate(['s', 'd']):
                    tmp = bp.tile([128, HF], F32, tag="btmp")
                    nc.vector.tensor_tensor(
                        out=tmp[:], in0=w1aug_s[:, 0:HF],
                        in1=a_s[:, i, :], op=ALU.mult)
                    red = bp.tile([128, H], F32, tag="bred")
                    nc.vector.tensor_reduce(
                        out=red[:], in_=tmp[:].rearrange("p (h f) -> p h f", h=H),
                        op=ALU.add, axis=AX)
                    if i == 0:
                        nc.vector.tensor_copy(out=w1aug_s[:, HF + F:WAUG1], in_=red[:])
                    else:
                        nc.vector.tensor_copy(out=bd1_s[:], in_=red[:])
                # B_s2 / B_d2 per chunk of Wg2
                for k in range(7):
                    for i, col0 in [(2, HF), (3, HF + H)]:
                        tmp = bp.tile([128, HF], F32, tag="btmp")
                        nc.vector.tensor_tensor(
                            out=tmp[:], in0=w2aug_s[:, k, 0:HF],
                            in1=a_s[:, i, :], op=ALU.mult)
                        red = bp.tile([128, H], F32, tag="bred")
                        nc.vector.tensor_reduce(
                            out=red[:], in_=tmp[:].rearrange("p (h f) -> p h f", h=H),
                            op=ALU.add, axis=AX)
                        nc.vector.tensor_copy(out=w2aug_s[:, k, col0:col0 + H], in_=red[:])
                # c2 = bg1 @ W2aug
                ps_c2 = bps.tile([1, W2AUG], F32, space="PSUM", tag="psc2")
                for k in range(7):
                    nc.tensor.matmul(out=ps_c2[:, 0:512], lhsT=bg1_s[:, k, :],
                                     rhs=w2aug_s[:, k, 0:512], start=(k == 0), stop=(k == 6))
                    nc.tensor.matmul(out=ps_c2[:, 512:W2AUG], lhsT=bg1_s[:, k, :],
                                     rhs=w2aug_s[:, k, 512:W2AUG], start=(k == 0), stop=(k == 6))
                nc.vector.tensor_copy(out=c2_s[:], in_=ps_c2[:])
                # adst1 per block
                for b in range(NBLK):
                    ps_a = bps.tile([128, H], F32, space="PSUM", tag="psa")
                    nc.tensor.matmul(out=ps_a[:], lhsT=xT_s[:, 128 * b:128 * (b + 1)],
                                     rhs=bd1_s[:], start=True, stop=True)
                    nc.vector.tensor_copy(out=adst1_s[:, b, :], in_=ps_a[:])

            # ---------------- phase 1: GAT1 + GCN1 edge loop
            with tc.tile_pool(name="p1", bufs=3) as p1, \
                 tc.tile_pool(name="p1g", bufs=2) as p1g, \
                 tc.tile_pool(name="p1s", bufs=2, space="PSUM") as p1s, \
                 tc.tile_pool(name="p1acc", bufs=1, space="PSUM") as p1acc:
                for b in range(NBLK):
                    xgt = p1g.tile([128, 1, EPB], F16, tag="xgt")
                    nc.gpsimd.dma_gather(
                        out_ap=xgt[:], in_ap=x16[:],
                        idxs_ap=ixx_s[:, b * (EPB // 16):(b + 1) * (EPB // 16)],
                        num_idxs=EPB, num_idxs_reg=EPB, elem_size=XROW, transpose=True,
                        single_packet=False)
                    s01t_b = p1g.tile([128, EPB], F16, tag="s01tb")
                    nc.sync.dma_start(out=s01t_b[:], in_=s01t_d[b])
                    scomb_b = p1g.tile([128, TPB * 256], F16, tag="scombb")
                    nc.sync.dma_start(out=scomb_b[:], in_=scomb_d[b])
                    ps_out = p1acc.tile([128, HF], F32, space="PSUM", tag="psout", name="psout")[:]
                    ps_s = p1acc.tile([128, H], F32, space="PSUM", tag="pss", name="pss")[:]
                    ps_gcn = p1acc.tile([128, F], F32, space="PSUM", tag="psgcn", name="psgcn")[:]
                    for k in range(TPB):
                        s01_t = scomb_b[:, k * 256:k * 256 + 128]
                        snrm_t = scomb_b[:, k * 256 + 128:k * 256 + 256]
                        lhs = xgt[:, 0, 128 * k:128 * (k + 1)]
                        ps1 = p1s.tile([128, WAUG1], F32, space="PSUM", tag="ps1")
                        nc.tensor.matmul(out=ps1[:, 0:512], lhsT=lhs,
                                         rhs=w1aug_s[:, 0:512], start=True, stop=True)
                        nc.tensor.matmul(out=ps1[:, 512:WAUG1], lhsT=lhs,
                                         rhs=w1aug_s[:, 512:WAUG1], start=True, stop=False)
                        nc.tensor.matmul(out=ps1[:, HF + F:WAUG1],
                                         lhsT=s01t_b[:, 128 * k:128 * (k + 1)],
                                         rhs=adst1_s[:, b, :], start=False, stop=True)
                        ex = exA if k % 2 == 0 else exB
                        lr02 = p1.tile([128, H], F32, tag="lr02")
                        nc.scalar.activation(out=lr02[:], in_=ps1[:, HF + F:WAUG1],
                                             func=AF.Copy, scale=0.2)
                        lr = p1.tile([128, H], F32, tag="lr")
                        nc.vector.tensor_tensor(out=lr[:], in0=ps1[:, HF + F:WAUG1],
                                                in1=lr02[:], op=ALU.max)
                        nc.scalar.activation(out=ex[:, 0:H], in_=lr[:], func=AF.Exp)
                        exv = p1.tile([128, HF + F], F16, tag="exv")
                        nc.vector.tensor_tensor(
                            out=exv[:].rearrange("p (h f) -> p h f", h=H + 1),
                            in0=ps1[:, 0:HF + F].rearrange("p (h f) -> p h f", h=H + 1),
                            in1=ex[:, :, None].to_broadcast([128, H + 1, F]),
                            op=ALU.mult)
                        nc.tensor.matmul(out=ps_s, lhsT=s01_t, rhs=ex[:, 0:H],
                                         start=(k == 0), stop=(k == TPB - 1))
                        nc.tensor.matmul(out=ps_out[:, 0:512], lhsT=s01_t,
                                         rhs=exv[:, 0:512], start=(k == 0), stop=(k == TPB - 1))
                        nc.tensor.matmul(out=ps_out[:, 512:HF], lhsT=s01_t,
                                         rhs=exv[:, 512:HF], start=(k == 0), stop=(k == TPB - 1))
                        nc.tensor.matmul(out=ps_gcn, lhsT=snrm_t,
                                         rhs=exv[:, HF:HF + F], start=(k == 0), stop=(k == TPB - 1))
                    rec = p1.tile([128, H], F32, tag="rec")
                    nc.vector.reciprocal(out=rec[:], in_=ps_s)
                    nc.vector.tensor_tensor(
                        out=x1loc_s[:, b, :].rearrange("p (h f) -> p h f", h=H),
                        in0=ps_out[:].rearrange("p (h f) -> p h f", h=H),
                        in1=rec[:, :, None].to_broadcast([128, H, F]),
                        op=ALU.mult)
                    nc.vector.tensor_copy(out=agg1_s[:, b, :], in_=ps_gcn)

            # ---------------- phase 2: x2 table, T2 table, collectives, conv
            with tc.tile_pool(name="p2", bufs=2) as p2:
              with tc.tile_pool(name="p2sa", bufs=2, space="PSUM") as p2s, \
                   tc.tile_pool(name="p2ta", bufs=2, space="PSUM") as p2t:
                # x2 table shard + allgather (early, small)
                for b in range(NBLK):
                    psT = p2t.tile([128, 128], F16, space="PSUM", tag="psT")
                    nc.tensor.transpose(out=psT[:F, :], in_=agg1_s[:, b, :],
                                        identity=ident_s[:])
                    x2lt = p2.tile([128, 128], F16, tag="x2lt")
                    nc.vector.tensor_scalar(out=x2lt[:F, :], in0=psT[:F, :],
                                            scalar1=bgcnc_s[:F, :], scalar2=None,
                                            op0=ALU.add)
                    ps_x2 = p2s.tile([128, F], F32, space="PSUM", tag="psx2")
                    nc.tensor.matmul(out=ps_x2[:], lhsT=x2lt[:F, :], rhs=wgcn_s[:F, :],
                                     start=True, stop=True)
                    nc.vector.tensor_copy(out=xstage[:, 0:F], in_=ps_x2[:])
                    nc.sync.dma_start(out=comb_shard[128 * b:128 * (b + 1), 0:XROW],
                                      in_=xstage[:])
              with tc.tile_pool(name="p2sb", bufs=2, space="PSUM") as p2s, \
                   tc.tile_pool(name="p2tb", bufs=2, space="PSUM") as p2t:
                # x1loc transposes -> x1t_s
                x1t_s = p2.tile([128, 7, NPC], F16, tag="x1t", bufs=1)
                nc.vector.memset(x1t_s[:], 0.0)
                for b in range(NBLK):
                    for fb in range(7):
                        c0, c1 = 128 * fb, min(128 * (fb + 1), HF)
                        psT = p2t.tile([128, 128], F16, space="PSUM", tag="psT")
                        nc.tensor.transpose(out=psT[:c1 - c0, :],
                                            in_=x1loc_s[:, b, c0:c1],
                                            identity=ident_s[:])
                        nc.vector.tensor_copy(
                            out=x1t_s[0:c1 - c0, fb, 128 * b:128 * (b + 1)],
                            in_=psT[:c1 - c0, :])
                # T2 build
                for b in range(NBLK):
                    ps_t2 = p2s.tile([128, W2AUG], F32, space="PSUM", tag="pst2")
                    for k in range(7):
                        nc.tensor.matmul(out=ps_t2[:, 0:512],
                                         lhsT=x1t_s[:, k, 128 * b:128 * (b + 1)],
                                         rhs=w2aug_s[:, k, 0:512], start=(k == 0), stop=False)
                        nc.tensor.matmul(out=ps_t2[:, 512:W2AUG],
                                         lhsT=x1t_s[:, k, 128 * b:128 * (b + 1)],
                                         rhs=w2aug_s[:, k, 512:W2AUG], start=(k == 0), stop=False)
                    nc.tensor.matmul(out=ps_t2[:, 0:512], lhsT=ones_s[:],
                                     rhs=c2_s[:, 0:512], start=False, stop=True)
                    nc.tensor.matmul(out=ps_t2[:, 512:W2AUG], lhsT=ones_s[:],
                                     rhs=c2_s[:, 512:W2AUG], start=False, stop=True)
                    nc.vector.tensor_copy(out=t2stage[:, 0:T2W], in_=ps_t2[:, 0:T2W])
                    nc.vector.tensor_copy(out=adst2_s[:, b, :], in_=ps_t2[:, T2W:W2AUG])
                    nc.sync.dma_start(out=comb_shard[128 * b:128 * (b + 1), XROW:CROW],
                                      in_=t2stage[:])
                nc.gpsimd.collective_compute(
                    "AllGather", ALU.bypass, replica_groups=RG,
                    ins=[comb_shard[:]], outs=[comb_full[:]])

              with tc.tile_pool(name="p2sc", bufs=1, space="PSUM") as p2s, \
                   tc.tile_pool(name="p2tc", bufs=2, space="PSUM") as p2t:
                # conv branch (runs during the collectives)
                twin_s = p2.tile([32, GPC, 608], F16, tag="twin", bufs=1)
                nc.sync.dma_start(out=twin_s[:], in_=twin_d[:])
                ps_ya = p2s.tile([GPC, 512], F32, space="PSUM", tag="psya")
                ps_yb = p2s.tile([GPC, 96], F32, space="PSUM", tag="psyb")
                for g in range(GPC):
                    nc.tensor.matmul(out=ps_ya[:], lhsT=wsel_s[:, g, :],
                                     rhs=twin_s[:, g, 0:512], start=(g == 0), stop=(g == GPC - 1))
                    nc.tensor.matmul(out=ps_yb[:], lhsT=wsel_s[:, g, :],
                                     rhs=twin_s[:, g, 512:608], start=(g == 0), stop=(g == GPC - 1))
                y_s = p2.tile([GPC, 608], F16, tag="ys")
                nc.vector.tensor_scalar(out=y_s[:, 0:512], in0=ps_ya[:],
                                        scalar1=bconv_s[:], scalar2=0.0,
                                        op0=ALU.add, op1=ALU.max)
                nc.vector.tensor_scalar(out=y_s[:, 512:608], in0=ps_yb[:],
                                        scalar1=bconv_s[:], scalar2=0.0,
                                        op0=ALU.add, op1=ALU.max)
                yt_s = pp.tile([128, 5, GPC], F16, tag="yt")
                nc.vector.memset(yt_s[:], 0.0)
                for i in range(5):
                    c0, c1 = 128 * i, min(128 * (i + 1), 608)
                    psT = p2t.tile([128, 128], F16, space="PSUM", tag="psT")
                    nc.tensor.transpose(out=psT[:c1 - c0, :GPC], in_=y_s[:, c0:c1],
                                        identity=ident_s[:GPC, :GPC])
                    nc.vector.tensor_copy(out=yt_s[0:c1 - c0, i, :], in_=psT[:c1 - c0, :GPC])
                ps_xt = p2s.tile([GPC, 256], F32, space="PSUM", tag="psxt")
                for i in range(5):
                    nc.tensor.matmul(out=ps_xt[:], lhsT=yt_s[:, i, :], rhs=wxt_s[:, i, :],
                                     start=(i == 0), stop=(i == 4))
                xt_s = p2.tile([GPC, 256], F16, tag="xts")
                nc.vector.tensor_tensor(out=xt_s[:], in0=ps_xt[:],
                                        in1=bias_s['bxt'][:],
                                        op=ALU.add)
                xtT_s = pp.tile([128, 2, GPC], F16, tag="xtT")
                for i in range(2):
                    psT = p2t.tile([128, 128], F16, space="PSUM", tag="psT")
                    nc.tensor.transpose(out=psT[:, :GPC], in_=xt_s[:, 128 * i:128 * (i + 1)],
                                        identity=ident_s[:GPC, :GPC])
                    nc.vector.tensor_copy(out=xtT_s[:, i, :], in_=psT[:, :GPC])

            # ---------------- phase 3: GAT2 + GCN2 edge loop
            with tc.tile_pool(name="p3", bufs=3) as p3, \
                 tc.tile_pool(name="p3g", bufs=2) as p3g, \
                 tc.tile_pool(name="p3s", bufs=2, space="PSUM") as p3s, \
                 tc.tile_pool(name="p3acc", bufs=1, space="PSUM") as p3acc:
                for b in range(NBLK):
                    v2g = p3g.tile([128, TPB, TROW], F16, tag="v2g")
                    nc.gpsimd.dma_gather(
                        out_ap=v2g[:], in_ap=comb_full[:, XROW:CROW],
                        idxs_ap=ixt2_s[:, b * (EPB // 16):(b + 1) * (EPB // 16)],
                        num_idxs=EPB, num_idxs_reg=EPB, elem_size=TROW, elem_step=CROW,
                        single_packet=False)
                    vxg = p3g.tile([128, TPB, XROW], F16, tag="vxg")
                    nc.gpsimd.dma_gather(
                        out_ap=vxg[:], in_ap=comb_full[:, 0:XROW],
                        idxs_ap=ixt2_s[:, b * (EPB // 16):(b + 1) * (EPB // 16)],
                        num_idxs=EPB, num_idxs_reg=EPB, elem_size=XROW, elem_step=CROW,
                        single_packet=False)
                    s01t_b = p3g.tile([128, EPB], F16, tag="s01tb3")
                    nc.sync.dma_start(out=s01t_b[:], in_=s01t_d[b])
                    scomb_b = p3g.tile([128, TPB * 256], F16, tag="scombb3")
                    nc.sync.dma_start(out=scomb_b[:], in_=scomb_d[b])
                    ps_out = p3acc.tile([128, HF], F32, space="PSUM", tag="psout3", name="psout3")[:]
                    ps_s = p3acc.tile([128, H], F32, space="PSUM", tag="pss3", name="pss3")[:]
                    ps_g2 = p3acc.tile([128, F], F32, space="PSUM", tag="psg2", name="psg2")[:]
                    for k in range(TPB):
                        s01_t = scomb_b[:, k * 256:k * 256 + 128]
                        snrm_t = scomb_b[:, k * 256 + 128:k * 256 + 256]
                        ps_l = p3s.tile([128, H], F32, space="PSUM", tag="psl")
                        nc.tensor.matmul(out=ps_l[:], lhsT=s01t_b[:, 128 * k:128 * (k + 1)],
                                         rhs=adst2_s[:, b, :], start=True, stop=False)
                        nc.tensor.matmul(out=ps_l[:], lhsT=ident_s[:],
                                         rhs=v2g[:, k, HF:T2W], start=False, stop=True)
                        ex = exA if k % 2 == 0 else exB
                        lr02 = p3.tile([128, H], F32, tag="lr023")
                        nc.scalar.activation(out=lr02[:], in_=ps_l[:], func=AF.Copy, scale=0.2)
                        lr = p3.tile([128, H], F32, tag="lr3")
                        nc.vector.tensor_tensor(out=lr[:], in0=ps_l[:], in1=lr02[:], op=ALU.max)
                        nc.scalar.activation(out=ex[:, 0:H], in_=lr[:], func=AF.Exp)
                        exv = p3.tile([128, HF], F16, tag="exv3")
                        nc.vector.tensor_tensor(
                            out=exv[:].rearrange("p (h f) -> p h f", h=H),
                            in0=v2g[:, k, 0:HF].rearrange("p (h f) -> p h f", h=H),
                            in1=ex[:, 0:H, None].to_broadcast([128, H, F]),
                            op=ALU.mult)
                        nc.tensor.matmul(out=ps_s, lhsT=s01_t, rhs=ex[:, 0:H],
                                         start=(k == 0), stop=(k == TPB - 1))
                        nc.tensor.matmul(out=ps_out[:, 0:512], lhsT=s01_t,
                                         rhs=exv[:, 0:512], start=(k == 0), stop=(k == TPB - 1))
                        nc.tensor.matmul(out=ps_out[:, 512:HF], lhsT=s01_t,
                                         rhs=exv[:, 512:HF], start=(k == 0), stop=(k == TPB - 1))
                        nc.tensor.matmul(out=ps_g2, lhsT=snrm_t,
                                         rhs=vxg[:, k, 0:F], start=(k == 0), stop=(k == TPB - 1))
                    rec = p3.tile([128, H], F32, tag="rec3")
                    nc.vector.reciprocal(out=rec[:], in_=ps_s)
                    u_s = p3.tile([128, HF], F16, tag="us")
                    nc.vector.tensor_tensor(
                        out=u_s[:].rearrange("p (h f) -> p h f", h=H),
                        in0=ps_out.rearrange("p (h f) -> p h f", h=H),
                        in1=rec[:, :, None].to_broadcast([128, H, F]),
                        op=ALU.mult)
                    v_s = p3.tile([128, HF], F16, tag="vs")
                    nc.vector.tensor_tensor(out=v_s[:], in0=u_s[:],
                                            in1=bg2_s[:],
                                            op=ALU.add)
                    nc.vector.tensor_scalar(out=t2stage[:, 0:HF], in0=v_s[:],
                                            scalar1=0.0, scalar2=None, op0=ALU.max)
                    nc.sync.dma_start(out=x1f_dram[128 * b:128 * (b + 1), :], in_=t2stage[:])
                    g2f = p3.tile([128, F], F32, tag="g2f")
                    nc.vector.tensor_tensor(out=g2f[:], in0=ps_g2,
                                            in1=bgcnr_s[:],
                                            op=ALU.add)
                    nc.vector.tensor_scalar(out=xstage[:, 0:F], in0=g2f[:],
                                            scalar1=0.0, scalar2=None, op0=ALU.max)
                    nc.sync.dma_start(out=x2f_dram[128 * b:128 * (b + 1), :], in_=xstage[:])

            # ---------------- phase 4: pooling + head
            with tc.tile_pool(name="p4", bufs=2) as p4:
              with tc.tile_pool(name="p4s", bufs=1, space="PSUM") as p4s:
                pass
                x1f_s = p4.tile([128, NBLK, HF], F16, tag="x1fp4", bufs=1)
                for b in range(NBLK):
                    nc.sync.dma_start(out=x1f_s[:, b, :],
                                      in_=x1f_dram[128 * b:128 * (b + 1), 0:HF])
                x2f_s = p4.tile([128, NBLK, F], F16, tag="x2fp4", bufs=1)
                for b in range(NBLK):
                    nc.sync.dma_start(out=x2f_s[:, b, :],
                                      in_=x2f_dram[128 * b:128 * (b + 1), 0:F])
                gmax1T = pp.tile([128, 7, GPC], F16, tag="gmax1T")
                gmax2T = pp.tile([128, 1, GPC], F16, tag="gmax2T")
                CH = GPC // 2
                for h in range(2):
                    slab = p4.tile([128, 7, CH * PW], F16, tag="slab")
                    nc.gpsimd.dma_gather(
                        out_ap=slab[:], in_ap=x1f_dram[:],
                        idxs_ap=ixp_s[:, h * (CH * PW // 16):(h + 1) * (CH * PW // 16)],
                        num_idxs=CH * PW, num_idxs_reg=CH * PW, elem_size=TROW,
                        transpose=True, single_packet=False)
                    for g in range(CH):
                        for j in range(7):
                            nc.vector.tensor_reduce(
                                out=gmax1T[:, j, h * CH + g:h * CH + g + 1],
                                in_=slab[:, j, g * PW:(g + 1) * PW],
                                op=ALU.max, axis=AX)
                    slab2 = p4.tile([128, 1, CH * PW], F16, tag="slab2")
                    nc.gpsimd.dma_gather(
                        out_ap=slab2[:], in_ap=x2f_dram[:],
                        idxs_ap=ixp_s[:, h * (CH * PW // 16):(h + 1) * (CH * PW // 16)],
                        num_idxs=CH * PW, num_idxs_reg=CH * PW, elem_size=XROW,
                        transpose=True, single_packet=False)
                    for g in range(CH):
                        nc.vector.tensor_reduce(
                            out=gmax2T[:, 0, h * CH + g:h * CH + g + 1],
                            in_=slab2[:, 0, g * PW:(g + 1) * PW],
                            op=ALU.max, axis=AX)
              # means via matmul, then transpose
              with tc.tile_pool(name="p4sm", bufs=1, space="PSUM") as p4s:
                ps_m1 = p4s.tile([GPC, HF], F32, space="PSUM", tag="psm1")
                ps_m2 = p4s.tile([GPC, F], F32, space="PSUM", tag="psm2")
                for b in range(NBLK):
                    nc.tensor.matmul(out=ps_m1[:, 0:512], lhsT=mmean_s[:, b, :],
                                     rhs=x1f_s[:, b, 0:512], start=(b == 0), stop=(b == NBLK - 1))
                    nc.tensor.matmul(out=ps_m1[:, 512:HF], lhsT=mmean_s[:, b, :],
                                     rhs=x1f_s[:, b, 512:HF], start=(b == 0), stop=(b == NBLK - 1))
                    nc.tensor.matmul(out=ps_m2[:], lhsT=mmean_s[:, b, :],
                                     rhs=x2f_s[:, b, :], start=(b == 0), stop=(b == NBLK - 1))
                mean1 = p4.tile([GPC, HF], F16, tag="mean1")
                nc.vector.tensor_copy(out=mean1[:], in_=ps_m1[:])
                mean2 = p4.tile([GPC, F], F16, tag="mean2")
                nc.vector.tensor_copy(out=mean2[:], in_=ps_m2[:])
              with tc.tile_pool(name="p4sh", bufs=1, space="PSUM") as p4s:
                gmean1T = pp.tile([128, 7, GPC], F16, tag="gmean1T")
                nc.vector.memset(gmean1T[:], 0.0)
                gmean2T = pp.tile([128, 1, GPC], F16, tag="gmean2T")
                nc.vector.memset(gmean2T[:], 0.0)
                for i in range(7):
                    c0, c1 = 128 * i, min(128 * (i + 1), HF)
                    psT = p4s.tile([128, 128], F16, space="PSUM", tag="psT4", bufs=2)
                    nc.tensor.transpose(out=psT[:c1 - c0, :GPC], in_=mean1[:, c0:c1],
                                        identity=ident_s[:GPC, :GPC])
                    nc.vector.tensor_copy(out=gmean1T[0:c1 - c0, i, :], in_=psT[:c1 - c0, :GPC])
                psT = p4s.tile([128, 128], F16, space="PSUM", tag="psT4", bufs=2)
                nc.tensor.transpose(out=psT[:F, :GPC], in_=mean2[:], identity=ident_s[:GPC, :GPC])
                nc.vector.tensor_copy(out=gmean2T[0:F, 0, :], in_=psT[:F, :GPC])

                def head_mm(ps, chunks, rhs_tile, nw):
                    n = len(chunks)
                    for i, ch in enumerate(chunks):
                        nc.tensor.matmul(out=ps[:], lhsT=ch, rhs=rhs_tile[:, i, :nw],
                                         start=(i == 0), stop=(i == n - 1))

                def bias_relu_T(ps, bias_ap, w, relu, nT, tagb):
                    zs = p4.tile([GPC, w], F16, tag="z" + tagb)
                    nc.vector.tensor_tensor(out=zs[:], in0=ps[:],
                                            in1=bias_ap, op=ALU.add)
                    if relu:
                        nc.vector.tensor_scalar(out=zs[:], in0=zs[:], scalar1=0.0,
                                                scalar2=None, op0=ALU.max)
                    zT = pp.tile([128, nT, GPC], F16, tag="zT" + tagb)
                    for i in range(nT):
                        psT2 = p4s.tile([128, 128], F16, space="PSUM", tag="psT4", bufs=2)
                        nc.tensor.transpose(out=psT2[:, :GPC], in_=zs[:, 128 * i:128 * (i + 1)],
                                            identity=ident_s[:GPC, :GPC])
                        nc.vector.tensor_copy(out=zT[:, i, :], in_=psT2[:, :GPC])
                    return zT

                ps_z1 = p4s.tile([GPC, 128], F32, space="PSUM", tag="psz1")
                head_mm(ps_z1, [gmax1T[:, j, :] for j in range(7)]
                        + [gmean1T[:, j, :] for j in range(7)], wfg1_s, 128)
                z1T = bias_relu_T(ps_z1, bias_s['bfg1'][:], 128, True, 1, "1")
                ps_z2 = p4s.tile([GPC, 128], F32, space="PSUM", tag="psz2")
                head_mm(ps_z2, [gmax2T[:, 0, :], gmean2T[:, 0, :]], wfg2_s, 128)
                z2T = bias_relu_T(ps_z2, bias_s['bfg2'][:], 128, True, 1, "2")
                ps_h1 = p4s.tile([GPC, 512], F32, space="PSUM", tag="psh1")
                head_mm(ps_h1, [z1T[:, 0, :], z2T[:, 0, :], xtT_s[:, 0, :], xtT_s[:, 1, :]],
                        w1_s, 512)
                h1T = bias_relu_T(ps_h1, bias_s['b1'][:], 512, True, 4, "h1")
                ps_h2 = p4s.tile([GPC, 256], F32, space="PSUM", tag="psh2")
                head_mm(ps_h2, [h1T[:, i, :] for i in range(4)], w2_s, 256)
                h2T = bias_relu_T(ps_h2, bias_s['b2'][:], 256, True, 2, "h2")
                ps_o = p4s.tile([GPC, 1], F32, space="PSUM", tag="pso")
                head_mm(ps_o, [h2T[:, i, :] for i in range(2)], wo_s, 1)
                o_s = p4.tile([GPC, 1], F32, tag="os")
                nc.vector.tensor_scalar(out=o_s[:], in0=ps_o[:], scalar1=bo_s[:],
                                        scalar2=None, op0=ALU.add)
                nc.sync.dma_start(out=out_d[:], in_=o_s[:])

    nc.compile()
    return nc


def build_in_maps(nc, shared, cores):
    declared = set()
    import concourse.mybir as _mb
    for alloc in nc.m.functions[0].allocations:
        if isinstance(alloc, _mb.MemoryLocationSet) and alloc.kind == "ExternalInput":
            declared.add(alloc.memorylocations[0].name)
    in_maps = []
    for c in range(8):
        m = dict(shared)
        m.update(cores[c])
        in_maps.append({k: np.ascontiguousarray(v) for k, v in m.items()
                        if k in declared})
    return in_maps


_CACHE = {}


def run_device(inputs):
    meta, shared, cores = prep(**inputs)
    key = (meta['NBLK'], meta['TPB'], meta['PW'])
    if key not in _CACHE:
        _CACHE[key] = build(meta)
    nc = _CACHE[key]
    in_maps = build_in_maps(nc, shared, cores)
    res = run_bass_kernel_spmd(nc, in_maps, core_ids=list(range(8)))
    out = np.concatenate([res.results[c]['out'] for c in range(8)], axis=0)
    return out.astype(np.float32)


def kernel(**inputs):
    return run_device(inputs)



# revision 2
# speedup vs baseline: 1.0668x; 1.0668x over previous
"""Trainium2 Bass kernel for nn_GAT_GCN (gnn_message_passing), 8 NeuronCores.

v2 strategy (restructured from baseline):
 - Projection commutes with the per-head softmax aggregation, so both GAT
   layers aggregate RAW per-edge features (x for layer 1, h2 for layer 2) and
   the projections are folded on the host: W2eff = blockdiag(Wg1) @ Wg2 etc.
   Phase 1 therefore needs no per-edge projection matmuls at all.
 - Per-edge alpha scaling uses a duplicated-exponent layout (ex_dup[128,H,2])
   so every TensorTensor operand is packed f16 SBUF -> DVE 2x mode.
 - leaky_relu(x) = 0.6x + 0.4|x| is fused into two Activation-engine ops.
 - One merged node table [h2 | asrc2 | xp] (896 cols) is exchanged with a
   single AllGather and gathered per edge once in phase 3.
 - Segment softmax/scatter-add remain matmuls against host-built 0/1
   selection blocks; GCN norm is applied per-edge via tensor_scalar.
"""
import sys
sys.path.insert(0, '/opt/trn_rl_repo')
import numpy as np

N, E, G, F, H = 16384, 131072, 128, 78, 10
NCORE, GPC = 8, 16          # cores, graphs per core
HF = H * F                  # 780
W2AUG = HF + 2 * H          # 800 = h2 | asrc2 | adst2
XROW = 128                  # xa table row, f16 (256B): [x 78 | asrc1 10 | pad]
CROW = 896                  # comb/x1f row, f16 (1792B): [h2 780 | asrc2 10 | xp 78 | pad]
AGGW = HF + F + H           # 868 = gat 780 | gcn 78 | den 10


def _wrap16(v):
    """dma_gather idx layout: [128, len/16] int16, idx i at (i%16, i//16),
    replicated across the 8 Q7 core groups."""
    v = np.asarray(v, np.int16)
    assert len(v) % 16 == 0
    m = v.reshape(-1, 16).T            # [16, S]
    return np.tile(m, (8, 1)).copy()   # [128, S]


def _f16(a):
    return np.ascontiguousarray(np.asarray(a, np.float32)).astype(np.float16)


def prep(x, edge_index, batch, target, Wg1, as1, ad1, bg1, Wg2, as2, ad2, bg2,
         Wgcn, bgcn, Wfg1, bfg1, Wfg2, bfg2, wconv, bconv, Wxt, bxt,
         W1, b1, W2, b2, Wo, bo):
    x = np.asarray(x, np.float64)
    ei = np.asarray(edge_index, np.int64)
    batch = np.asarray(batch, np.int64)
    target = np.asarray(target, np.float32)
    Wg1 = np.asarray(Wg1, np.float64); Wg2 = np.asarray(Wg2, np.float64)
    as1 = np.asarray(as1, np.float64); ad1 = np.asarray(ad1, np.float64)
    as2 = np.asarray(as2, np.float64); ad2 = np.asarray(ad2, np.float64)
    bg1 = np.asarray(bg1, np.float64)
    Wgcn = np.asarray(Wgcn, np.float64); bgcn_ = np.asarray(bgcn, np.float64)

    loops = np.arange(N, dtype=np.int64)
    src = np.concatenate([ei[0], loops])
    dst = np.concatenate([ei[1], loops])

    # ---- host-folded weights
    B_s1 = np.zeros((F, H)); B_d1 = np.zeros((F, H))
    for h in range(H):
        B_s1[:, h] = Wg1[:, h * F:(h + 1) * F] @ as1[h]
        B_d1[:, h] = Wg1[:, h * F:(h + 1) * F] @ ad1[h]
    asrc1 = x @ B_s1                                  # [N, H]
    adst1 = x @ B_d1                                  # [N, H]
    Wt1 = np.zeros((HF, HF))
    for h in range(H):
        Wt1[h * F:(h + 1) * F, h * F:(h + 1) * F] = Wg1[:, h * F:(h + 1) * F]
    B_s2 = (Wg2.reshape(HF, H, F) * as2[:, None, :].transpose(1, 0, 2)).sum(-1)
    B_d2 = (Wg2.reshape(HF, H, F) * ad2[:, None, :].transpose(1, 0, 2)).sum(-1)
    Wcat = np.concatenate([Wg2, B_s2, B_d2], axis=1)  # [780, 800]
    W2effaug = Wt1 @ Wcat                             # [780, 800]
    c2aug = bg1 @ Wcat                                # [800]
    Wgcn2 = Wgcn @ Wgcn
    cgcn = bgcn_ @ Wgcn

    # graph-aligned core boundaries
    counts = np.bincount(batch, minlength=G)
    node_off = np.concatenate([[0], np.cumsum(counts)])
    n_lo = node_off[np.arange(NCORE) * GPC]
    n_hi = node_off[(np.arange(NCORE) + 1) * GPC]

    # degrees / gcn norm (over full edge list incl self loops)
    deg = np.bincount(dst, minlength=N).astype(np.float64)
    dinv = 1.0 / np.sqrt(deg)
    norm = (dinv[src] * dinv[dst]).astype(np.float32)

    order = np.argsort(dst, kind='stable')
    srcs, dsts, norms = src[order], dst[order], norm[order]

    Lmax = int((n_hi - n_lo).max())
    NBLK = (Lmax + 127) // 128
    NPC = NBLK * 128
    assert NCORE * NPC < 32768

    node_owner = np.searchsorted(n_hi - 1, np.arange(N), side='left')
    node_owner = np.minimum(node_owner, NCORE - 1)
    pad_gid = node_owner * NPC + (np.arange(N) - n_lo[node_owner])

    # per (core, block) edge spans -> uniform TPB
    spans = []
    TPB = 1
    for c in range(NCORE):
        e1 = np.searchsorted(dsts, n_hi[c])
        bl = []
        for b in range(NBLK):
            lo = np.searchsorted(dsts, n_lo[c] + 128 * b)
            hi = np.searchsorted(dsts, min(n_lo[c] + 128 * (b + 1), n_hi[c]))
            if n_lo[c] + 128 * b >= n_hi[c]:
                lo = hi = e1
            bl.append((lo, hi))
            TPB = max(TPB, (hi - lo + 127) // 128)
        spans.append(bl)
    ET = NBLK * TPB
    ECAP = ET * 128
    TPBb = []
    for b in range(NBLK):
        m = 1
        for c in range(NCORE):
            lo, hi = spans[c][b]
            m = max(m, (hi - lo + 127) // 128)
        TPBb.append(int(m))

    PW = int(np.ceil(counts.max() / 16) * 16)   # pool slot width per graph

    cores = []
    for c in range(NCORE):
        esrc = np.zeros(ECAP, np.int64)           # raw src id per edge slot
        s01 = np.zeros((NBLK, 128, TPB * 128), np.float16)
        s01t = np.zeros((NBLK, 128, TPB * 128), np.float16)
        normv = np.zeros((NBLK, 128, TPB), np.float32)
        for b in range(NBLK):
            lo, hi = spans[c][b]
            ne = hi - lo
            if ne == 0:
                continue
            sl = slice(b * TPB * 128, b * TPB * 128 + ne)
            esrc[sl] = srcs[lo:hi]
            ld = (dsts[lo:hi] - n_lo[c] - 128 * b).astype(np.int64)  # 0..127
            j = np.arange(ne)
            t_loc = j // 128
            e_loc = j % 128
            s01[b, e_loc, t_loc * 128 + ld] = 1.0
            s01t[b, ld, j] = 1.0
            normv[b, e_loc, t_loc] = norms[lo:hi]
        # padded dst columns (no incoming edges) get one dummy S entry so the
        # softmax denominator stays finite (their rows are garbage, never read)
        Lc = int(n_hi[c] - n_lo[c])
        for b in range(NBLK):
            first_pad = max(0, min(128, Lc - 128 * b))
            if first_pad < 128:
                s01[b, 0, first_pad:128] = 1.0

        # adst1 for local dst nodes
        adst1loc = np.zeros((NBLK, 128, H), np.float16)
        av = adst1[n_lo[c]:n_hi[c]].astype(np.float16)
        ids = np.arange(Lc)
        adst1loc[ids // 128, ids % 128, :] = av

        # pooling indices (local node ids into x1f/x2f tables)
        pool_idx = np.zeros(GPC * PW, np.int64)
        for g in range(GPC):
            gg = c * GPC + g
            a, bnd = node_off[gg] - n_lo[c], node_off[gg + 1] - n_lo[c]
            cnt = bnd - a
            pool_idx[g * PW:g * PW + cnt] = np.arange(a, bnd)
            pool_idx[g * PW + cnt:(g + 1) * PW] = a      # pad = first node of graph
        mmean = np.zeros((NBLK, 128, GPC), np.float16)
        for g in range(GPC):
            gg = c * GPC + g
            a, bnd = node_off[gg] - n_lo[c], node_off[gg + 1] - n_lo[c]
            ids = np.arange(a, bnd)
            mmean[ids // 128, ids % 128, g] = np.float16(1.0 / (bnd - a))

        # conv im2col: [32, GPC, 608]
        t_win = np.zeros((32, GPC, 608), np.float16)
        tg = target[c * GPC:(c + 1) * GPC, 0, :]          # [GPC, 625]
        for k in range(32):
            t_win[k, :, :594] = tg[:, k:k + 594].astype(np.float16)

        cores.append(dict(
            ix_x=_wrap16(esrc),                     # for xa-gather (raw ids)
            ix_t2=_wrap16(pad_gid[esrc]),           # for comb gathers (padded ids)
            ix_pool=_wrap16(pool_idx),
            s01=s01, s01t=s01t, normv=normv,
            adst1loc=np.ascontiguousarray(adst1loc.transpose(1, 0, 2)),
            mmean=np.ascontiguousarray(mmean.transpose(1, 0, 2)), t_win=t_win,
            bconv_rep=np.full((GPC, 1), float(np.asarray(bconv).reshape(-1)[0]), np.float32),
        ))

    # gather table: [x | asrc1]
    xa16 = np.zeros((N, XROW), np.float16)
    xa16[:, :F] = x.astype(np.float16)
    xa16[:, F:F + H] = asrc1.astype(np.float16)

    # W2effaug split into 7 chunks of 128 rows over a [896, 800] matrix whose
    # rows 780:896 are zero (transposed aggregate cols 780+ are gcn/garbage)
    w2eff = np.zeros((7, 128, W2AUG), np.float16)
    for k in range(7):
        r0, r1 = 128 * k, min(128 * (k + 1), HF)
        w2eff[k, :r1 - r0, :] = W2effaug[r0:r1].astype(np.float16)

    wgcn2_s = np.zeros((128, F), np.float16)
    wgcn2_s[:F] = Wgcn2.astype(np.float16)

    def pack_rows(Wm, splits, ncol):
        out = np.zeros((len(splits), 128, ncol), np.float16)
        for i, (r0, r1) in enumerate(splits):
            out[i, :r1 - r0, :] = _f16(Wm[r0:r1, :])
        return out

    sp7 = [(128 * i, min(128 * (i + 1), HF)) for i in range(7)]
    wfg1p = np.concatenate([pack_rows(np.asarray(Wfg1)[:HF], sp7, 128),
                            pack_rows(np.asarray(Wfg1)[HF:], sp7, 128)], axis=0)
    wfg2p = pack_rows(np.asarray(Wfg2), [(0, F), (F, 2 * F)], 128)
    wxtp = pack_rows(np.asarray(Wxt), [(128 * i, min(128 * (i + 1), 594)) for i in range(5)], 256)
    w1p = pack_rows(np.asarray(W1), [(128 * i, 128 * (i + 1)) for i in range(4)], 512)
    w2p = pack_rows(np.asarray(W2), [(128 * i, 128 * (i + 1)) for i in range(4)], 256)
    wop = pack_rows(np.asarray(Wo), [(0, 128), (128, 256)], 1)

    shared = dict(
        xa16=xa16, w2eff=np.ascontiguousarray(w2eff.transpose(1, 0, 2)),
        c2row=c2aug.astype(np.float16).reshape(1, W2AUG),
        wgcn2_s=wgcn2_s,
        cgcn_row=cgcn.astype(np.float32).reshape(1, F),
        bgcn_row=np.asarray(bgcn, np.float32).reshape(1, F),
        bg2row=np.asarray(bg2, np.float16).reshape(1, HF),
        wfg1p=np.ascontiguousarray(wfg1p.transpose(1, 0, 2)),
        bfg1=np.asarray(bfg1, np.float32).reshape(1, 128),
        wfg2p=np.ascontiguousarray(wfg2p.transpose(1, 0, 2)),
        bfg2=np.asarray(bfg2, np.float32).reshape(1, 128),
        wxtp=np.ascontiguousarray(wxtp.transpose(1, 0, 2)),
        bxt=np.asarray(bxt, np.float32).reshape(1, 256),
        w1p=np.ascontiguousarray(w1p.transpose(1, 0, 2)),
        b1=np.asarray(b1, np.float32).reshape(1, 512),
        w2p=np.ascontiguousarray(w2p.transpose(1, 0, 2)),
        b2=np.asarray(b2, np.float32).reshape(1, 256),
        wop=np.ascontiguousarray(wop.transpose(1, 0, 2)), bo_rep=np.full((GPC, 1), float(np.asarray(bo).reshape(-1)[0]), np.float32),
        w_sel=np.zeros((32, GPC, GPC), np.float16),
    )
    wcol = _f16(np.asarray(wconv).reshape(-1))
    for g in range(GPC):
        shared['w_sel'][:, g, g] = wcol

    meta = dict(NBLK=NBLK, NPC=NPC, TPB=TPB, ET=ET, ECAP=ECAP, PW=PW,
                TPBb=tuple(TPBb), n_lo=n_lo, n_hi=n_hi)
    return meta, shared, cores


import concourse.bass as bass
import concourse.bacc as bacc
import concourse.mybir as mybir
from concourse import library_config
from concourse.tile import TileContext
from concourse.masks import make_identity
from concourse.bass_utils import run_bass_kernel_spmd

F16 = mybir.dt.float16
F32 = mybir.dt.float32
I16 = mybir.dt.int16
AX = mybir.AxisListType.X
ALU = mybir.AluOpType
AF = mybir.ActivationFunctionType


def build(meta):
    NBLK, NPC, TPB, ET, ECAP, PW = (meta[k] for k in
                                    ['NBLK', 'NPC', 'TPB', 'ET', 'ECAP', 'PW'])
    TPBb = meta['TPBb']
    EPB = TPB * 128                       # edges per block
    nc = bacc.Bacc()

    dp = lambda n, s, d: nc.declare_dram_parameter(n, list(s), d, isOutput=False)
    # per-core inputs
    xa16 = dp('xa16', [N, XROW], F16)
    ix_x = dp('ix_x', [128, ECAP // 16], I16)
    ix_t2 = dp('ix_t2', [128, ECAP // 16], I16)
    ix_pool = dp('ix_pool', [128, GPC * PW // 16], I16)
    s01_d = dp('s01', [NBLK, 128, EPB], F16)
    s01t_d = dp('s01t', [NBLK, 128, EPB], F16)
    normv_d = dp('normv', [NBLK, 128, TPB], F32)
    adst1_d = dp('adst1loc', [128, NBLK, H], F16)
    mmean_d = dp('mmean', [128, NBLK, GPC], F16)
    twin_d = dp('t_win', [32, GPC, 608], F16)
    bconv_rep = dp('bconv_rep', [GPC, 1], F32)
    # shared weights
    w2eff_d = dp('w2eff', [128, 7, W2AUG], F16)
    c2row = dp('c2row', [1, W2AUG], F16)
    wgcn2 = dp('wgcn2_s', [128, F], F16)
    cgcn_row = dp('cgcn_row', [1, F], F32)
    bgcn_row = dp('bgcn_row', [1, F], F32)
    bg2row = dp('bg2row', [1, HF], F16)
    wfg1p = dp('wfg1p', [128, 14, 128], F16)
    bfg1 = dp('bfg1', [1, 128], F32)
    wfg2p = dp('wfg2p', [128, 2, 128], F16)
    bfg2 = dp('bfg2', [1, 128], F32)
    wxtp = dp('wxtp', [128, 5, 256], F16)
    bxt = dp('bxt', [1, 256], F32)
    w1p = dp('w1p', [128, 4, 512], F16)
    b1 = dp('b1', [1, 512], F32)
    w2p = dp('w2p', [128, 4, 256], F16)
    b2 = dp('b2', [1, 256], F32)
    wop = dp('wop', [128, 2, 1], F16)
    bo_rep = dp('bo_rep', [GPC, 1], F32)
    wsel_d = dp('w_sel', [32, GPC, GPC], F16)

    out_d = nc.declare_dram_parameter('out', [GPC, 1], F32, isOutput=True)

    # internal DRAM
    comb_shard = nc.dram_tensor('comb_shard', [NPC, CROW], F16)
    comb_full = nc.dram_tensor('comb_full', [8 * NPC, CROW], F16, addr_space="Shared")
    x1f_dram = nc.dram_tensor('x1f_dram', [NPC, CROW], F16)
    x2f_dram = nc.dram_tensor('x2f_dram', [NPC, XROW], F16)

    RG = [list(range(8))]

    with TileContext(nc) as tc:
        nc.gpsimd.load_library(library_config.mlp)

        with tc.tile_pool(name="persist", bufs=1) as pp:
            # ---------------- persistent tiles + loads
            adst1_s = pp.tile([128, NBLK, H], F16, tag="adst1")
            nc.sync.dma_start(out=adst1_s[:], in_=adst1_d[:])
            w2eff_s = pp.tile([128, 7, W2AUG], F16, tag="w2eff")
            nc.sync.dma_start(out=w2eff_s[:], in_=w2eff_d[:])
            c2_s = pp.tile([1, W2AUG], F16, tag="c2")
            nc.sync.dma_start(out=c2_s[:], in_=c2row[:])
            wgcn2_s = pp.tile([128, F], F16, tag="wgcn2")
            nc.sync.dma_start(out=wgcn2_s[:], in_=wgcn2[:])
            cgcn_s = pp.tile([128, F], F32, tag="cgcn")
            nc.sync.dma_start(out=cgcn_s[:], in_=cgcn_row[:].to_broadcast([128, F]))
            bgcnr_s = pp.tile([128, F], F32, tag="bgcnr")
            nc.sync.dma_start(out=bgcnr_s[:], in_=bgcn_row[:].to_broadcast([128, F]))
            bg2_s = pp.tile([128, HF], F16, tag="bg2")
            nc.sync.dma_start(out=bg2_s[:], in_=bg2row[:].to_broadcast([128, HF]))
            ixx_s = pp.tile([128, ECAP // 16], I16, tag="ixx")
            nc.sync.dma_start(out=ixx_s[:], in_=ix_x[:])
            ixt2_s = pp.tile([128, ECAP // 16], I16, tag="ixt2")
            nc.sync.dma_start(out=ixt2_s[:], in_=ix_t2[:])
            ixp_s = pp.tile([128, GPC * PW // 16], I16, tag="ixp")
            nc.sync.dma_start(out=ixp_s[:], in_=ix_pool[:])
            mmean_s = pp.tile([128, NBLK, GPC], F16, tag="mmean")
            nc.sync.dma_start(out=mmean_s[:], in_=mmean_d[:])
            wsel_s = pp.tile([32, GPC, GPC], F16, tag="wsel")
            nc.sync.dma_start(out=wsel_s[:], in_=wsel_d[:])
            bconv_s = pp.tile([GPC, 1], F32, tag="bconv")
            nc.sync.dma_start(out=bconv_s[:], in_=bconv_rep[:])
            wfg1_s = pp.tile([128, 14, 128], F16, tag="wfg1")
            nc.sync.dma_start(out=wfg1_s[:], in_=wfg1p[:])
            wfg2_s = pp.tile([128, 2, 128], F16, tag="wfg2")
            nc.sync.dma_start(out=wfg2_s[:], in_=wfg2p[:])
            wxt_s = pp.tile([128, 5, 256], F16, tag="wxt")
            nc.sync.dma_start(out=wxt_s[:], in_=wxtp[:])
            w1_s = pp.tile([128, 4, 512], F16, tag="w1")
            nc.sync.dma_start(out=w1_s[:], in_=w1p[:])
            w2_s = pp.tile([128, 4, 256], F16, tag="w2")
            nc.sync.dma_start(out=w2_s[:], in_=w2p[:])
            wo_s = pp.tile([128, 2, 1], F16, tag="wo")
            nc.sync.dma_start(out=wo_s[:], in_=wop[:])
            bias_s = {}
            for nm, t, w in [('bfg1', bfg1, 128), ('bfg2', bfg2, 128),
                             ('bxt', bxt, 256), ('b1', b1, 512), ('b2', b2, 256)]:
                bias_s[nm] = pp.tile([GPC, w], F32, tag="bias_" + nm, name="bias_" + nm)
                nc.sync.dma_start(out=bias_s[nm][:], in_=t[:].to_broadcast([GPC, w]))
            bo_s = pp.tile([GPC, 1], F32, tag="bo")
            nc.sync.dma_start(out=bo_s[:], in_=bo_rep[:])

            ident_s = pp.tile([128, 128], F16, tag="ident")
            make_identity(nc, ident_s[:])
            ones_s = pp.tile([1, 128], F16, tag="ones")
            nc.vector.memset(ones_s[:], 1.0)

            # work state
            adst2_s = pp.tile([128, NBLK, H], F16, tag="adst2")
            x1loc_s = pp.tile([128, NBLK, CROW], F16, tag="x1loc")
            nc.vector.memset(x1loc_s[:, :, HF + F:CROW], 0.0)
            t2stage = pp.tile([128, CROW], F16, tag="t2stage")
            nc.vector.memset(t2stage[:], 0.0)
            x1f_s = pp.tile([128, NBLK, HF], F16, tag="x1fs")
            x2f_s = pp.tile([128, NBLK, F], F16, tag="x2fs")
            # zero-fill the pad columns of the pooled-row tables once (the
            # pool gather reads whole rows; uninit DRAM is NaN in the sim)
            zpad = pp.tile([128, NBLK, CROW - HF], F16, tag="zpad")
            nc.vector.memset(zpad[:], 0.0)
            nc.sync.dma_start(
                out=x1f_dram[:].rearrange("(b p) c -> p b c", p=128)[:, :, HF:CROW],
                in_=zpad[:])
            nc.sync.dma_start(
                out=x2f_dram[:].rearrange("(b p) c -> p b c", p=128)[:, :, F:XROW],
                in_=zpad[:, :, 0:XROW - F])

            def edge_tile(p, ps, xg, s01_b, s01t_b, normb, adst_ap, b, k, src_off):
                """Per-edge-tile softmax + aggregation. xg rows are
                [payload | asrc (H at src_off) | ...]; payload cols 0:780 for
                GAT (broadcast over heads in phase 1), gcn payload at gcn_off.
                Returns nothing; accumulates into ps_agg (caller-held)."""
                pass  # logic inlined below per phase (layouts differ)

            # ---------------- phase 1: GAT1 + GCN1 edge loop (raw-x aggregation)
            with tc.tile_pool(name="p1", bufs=3) as p1, \
                 tc.tile_pool(name="p1g", bufs=2) as p1g, \
                 tc.tile_pool(name="p1s", bufs=3, space="PSUM") as p1s, \
                 tc.tile_pool(name="p1acc", bufs=2, space="PSUM") as p1acc:
                for b in range(NBLK):
                    nt = TPBb[b]
                    ne = 128 * nt
                    xg = p1g.tile([128, TPB, XROW], F16, tag="xg")
                    nc.gpsimd.dma_gather(
                        out_ap=xg[:, 0:nt, :], in_ap=xa16[:],
                        idxs_ap=ixx_s[:, b * (EPB // 16):b * (EPB // 16) + ne // 16],
                        num_idxs=ne, num_idxs_reg=ne, elem_size=XROW,
                        single_packet=False)
                    s01_b = p1g.tile([128, EPB], F16, tag="s01b")
                    nc.sync.dma_start(out=s01_b[:], in_=s01_d[b])
                    s01t_b = p1g.tile([128, EPB], F16, tag="s01tb")
                    nc.sync.dma_start(out=s01t_b[:], in_=s01t_d[b])
                    normb = p1g.tile([128, TPB], F32, tag="normb")
                    nc.sync.dma_start(out=normb[:], in_=normv_d[b])
                    ps_agg = p1acc.tile([128, AGGW], F32, space="PSUM",
                                        tag="psagg", name="psagg")[:]
                    for k in range(nt):
                        s01_t = s01_b[:, 128 * k:128 * (k + 1)]
                        ps_l = p1s.tile([128, H], F32, space="PSUM", tag="psl")
                        nc.tensor.matmul(out=ps_l[:], lhsT=s01t_b[:, 128 * k:128 * (k + 1)],
                                         rhs=adst1_s[:, b, :], start=True, stop=False)
                        nc.tensor.matmul(out=ps_l[:], lhsT=ident_s[:],
                                         rhs=xg[:, k, F:F + H], start=False, stop=True)
                        # exp(leaky_relu(lg)), lg = max(lg, 0.2*lg)
                        lr02 = p1.tile([128, H], F32, tag="lr02")
                        nc.scalar.activation(out=lr02[:], in_=ps_l[:], func=AF.Copy, scale=0.2)
                        lr = p1.tile([128, H], F32, tag="lr")
                        nc.vector.tensor_tensor(out=lr[:], in0=ps_l[:], in1=lr02[:], op=ALU.max)
                        ex_dup = p1.tile([128, H, 2], F16, tag="exdup")
                        exv = p1.tile([128, AGGW], F16, tag="exv")
                        nc.scalar.activation(out=exv[:, HF + F:AGGW], in_=lr[:],
                                             func=AF.Exp)
                        nc.vector.tensor_copy(
                            out=ex_dup[:],
                            in_=exv[:, HF + F:AGGW].rearrange(
                                "p (h one) -> p h one", one=1).to_broadcast([128, H, 2]))
                        # exv[:, 0:780] = x broadcast over heads * ex
                        nc.vector.tensor_tensor(
                            out=exv[:, 0:HF].rearrange("p (h f2 two) -> p h f2 two",
                                                       h=H, two=2),
                            in0=xg[:, k, 0:F].rearrange("p (f2 two) -> p f2 two", two=2)
                                [:, None, :, :].to_broadcast([128, H, F // 2, 2]),
                            in1=ex_dup[:, :, None, :].to_broadcast([128, H, F // 2, 2]),
                            op=ALU.mult)
                        # exv[:, 780:858] = x * norm (per-partition scalar)
                        nc.vector.tensor_scalar(
                            out=exv[:, HF:HF + F], in0=xg[:, k, 0:F],
                            scalar1=normb[:, k:k + 1], scalar2=None, op0=ALU.mult)
                        nc.tensor.matmul(out=ps_agg[:, 0:512], lhsT=s01_t,
                                         rhs=exv[:, 0:512], start=(k == 0), stop=(k == nt - 1))
                        nc.tensor.matmul(out=ps_agg[:, 512:AGGW], lhsT=s01_t,
                                         rhs=exv[:, 512:AGGW], start=(k == 0), stop=(k == nt - 1))
                    # normalize gat part per head; copy gcn part
                    rec = p1.tile([128, H], F32, tag="rec")
                    nc.vector.reciprocal(out=rec[:], in_=ps_agg[:, HF + F:AGGW])
                    rec_dup = p1.tile([128, H, 2], F32, tag="recdup")
                    nc.vector.tensor_copy(out=rec_dup[:],
                                          in_=rec[:, :, None].to_broadcast([128, H, 2]))
                    nc.vector.tensor_tensor(
                        out=x1loc_s[:, b, 0:HF].rearrange("p (h f2 two) -> p h f2 two",
                                                          h=H, two=2),
                        in0=ps_agg[:, 0:HF].rearrange("p (h f2 two) -> p h f2 two",
                                                      h=H, two=2),
                        in1=rec_dup[:, :, None, :].to_broadcast([128, H, F // 2, 2]),
                        op=ALU.mult)
                    nc.scalar.activation(out=x1loc_s[:, b, HF:HF + F],
                                         in_=ps_agg[:, HF:HF + F], func=AF.Copy)

            # ---------------- phase 2: T2 table (folded proj), collective, conv
            with tc.tile_pool(name="p2", bufs=2) as p2:
              with tc.tile_pool(name="p2sb", bufs=2, space="PSUM") as p2s, \
                   tc.tile_pool(name="p2tb", bufs=2, space="PSUM") as p2t:
                # aggn transposes -> x1t_s  [chunk r, 7, node]; gcn agg gets its
                # own partition-0-aligned transpose
                x1t_s = p2.tile([128, 7, NPC], F16, tag="x1t", bufs=1)
                gcnT_s = p2.tile([128, NPC], F16, tag="gcnT", bufs=1)
                for b in range(NBLK):
                    for fb in range(7):
                        psT = p2t.tile([128, 128], F16, space="PSUM", tag="psT")
                        nc.tensor.transpose(out=psT[:],
                                            in_=x1loc_s[:, b, 128 * fb:128 * (fb + 1)],
                                            identity=ident_s[:])
                        nc.vector.tensor_copy(
                            out=x1t_s[:, fb, 128 * b:128 * (b + 1)],
                            in_=psT[:])
                    psT = p2t.tile([128, 128], F16, space="PSUM", tag="psT")
                    nc.tensor.transpose(out=psT[:F, :],
                                        in_=x1loc_s[:, b, HF:HF + F],
                                        identity=ident_s[:])
                    nc.vector.tensor_copy(out=gcnT_s[0:F, 128 * b:128 * (b + 1)],
                                          in_=psT[:F, :])
                # T2 build + xp (gcn) per block
                for b in range(NBLK):
                    ps_t2 = p2s.tile([128, W2AUG], F32, space="PSUM", tag="pst2")
                    for k in range(7):
                        nc.tensor.matmul(out=ps_t2[:, 0:512],
                                         lhsT=x1t_s[:, k, 128 * b:128 * (b + 1)],
                                         rhs=w2eff_s[:, k, 0:512], start=(k == 0), stop=False)
                        nc.tensor.matmul(out=ps_t2[:, 512:W2AUG],
                                         lhsT=x1t_s[:, k, 128 * b:128 * (b + 1)],
                                         rhs=w2eff_s[:, k, 512:W2AUG], start=(k == 0), stop=False)
                    nc.tensor.matmul(out=ps_t2[:, 0:512], lhsT=ones_s[:],
                                     rhs=c2_s[:, 0:512], start=False, stop=True)
                    nc.tensor.matmul(out=ps_t2[:, 512:W2AUG], lhsT=ones_s[:],
                                     rhs=c2_s[:, 512:W2AUG], start=False, stop=True)
                    # xp = agg_gcn @ Wgcn^2 + cgcn ; agg_gcn rows are chunk-6
                    # local rows 12:90 of the transposed aggregate
                    ps_xp = p2s.tile([128, F], F32, space="PSUM", tag="psxp")
                    nc.tensor.matmul(out=ps_xp[:],
                                     lhsT=gcnT_s[0:F, 128 * b:128 * (b + 1)],
                                     rhs=wgcn2_s[0:F, :], start=True, stop=True)
                    nc.scalar.activation(out=t2stage[:, 0:HF + H], in_=ps_t2[:, 0:HF + H],
                                         func=AF.Copy)
                    nc.vector.tensor_copy(out=adst2_s[:, b, :], in_=ps_t2[:, HF + H:W2AUG])
                    nc.vector.tensor_tensor(out=t2stage[:, HF + H:HF + H + F],
                                            in0=ps_xp[:], in1=cgcn_s[:], op=ALU.add)
                    nc.sync.dma_start(out=comb_shard[128 * b:128 * (b + 1), :],
                                      in_=t2stage[:])
                nc.gpsimd.collective_compute(
                    "AllGather", ALU.bypass, replica_groups=RG,
                    ins=[comb_shard[:]], outs=[comb_full[:]])

              with tc.tile_pool(name="p2sc", bufs=1, space="PSUM") as p2s, \
                   tc.tile_pool(name="p2tc", bufs=2, space="PSUM") as p2t:
                # conv branch (runs during the collective)
                twin_s = p2.tile([32, GPC, 608], F16, tag="twin", bufs=1)
                nc.sync.dma_start(out=twin_s[:], in_=twin_d[:])
                ps_ya = p2s.tile([GPC, 512], F32, space="PSUM", tag="psya")
                ps_yb = p2s.tile([GPC, 96], F32, space="PSUM", tag="psyb")
                for g in range(GPC):
                    nc.tensor.matmul(out=ps_ya[:], lhsT=wsel_s[:, g, :],
                                     rhs=twin_s[:, g, 0:512], start=(g == 0), stop=(g == GPC - 1))
                    nc.tensor.matmul(out=ps_yb[:], lhsT=wsel_s[:, g, :],
                                     rhs=twin_s[:, g, 512:608], start=(g == 0), stop=(g == GPC - 1))
                y_s = p2.tile([GPC, 608], F16, tag="ys")
                nc.vector.tensor_scalar(out=y_s[:, 0:512], in0=ps_ya[:],
                                        scalar1=bconv_s[:], scalar2=0.0,
                                        op0=ALU.add, op1=ALU.max)
                nc.vector.tensor_scalar(out=y_s[:, 512:608], in0=ps_yb[:],
                                        scalar1=bconv_s[:], scalar2=0.0,
                                        op0=ALU.add, op1=ALU.max)
                yt_s = pp.tile([128, 5, GPC], F16, tag="yt")
                nc.vector.memset(yt_s[:], 0.0)
                for i in range(5):
                    c0, c1 = 128 * i, min(128 * (i + 1), 608)
                    psT = p2t.tile([128, 128], F16, space="PSUM", tag="psT")
                    nc.tensor.transpose(out=psT[:c1 - c0, :GPC], in_=y_s[:, c0:c1],
                                        identity=ident_s[:GPC, :GPC])
                    nc.vector.tensor_copy(out=yt_s[0:c1 - c0, i, :], in_=psT[:c1 - c0, :GPC])
                ps_xt = p2s.tile([GPC, 256], F32, space="PSUM", tag="psxt")
                for i in range(5):
                    nc.tensor.matmul(out=ps_xt[:], lhsT=yt_s[:, i, :], rhs=wxt_s[:, i, :],
                                     start=(i == 0), stop=(i == 4))
                xt_s = p2.tile([GPC, 256], F16, tag="xts")
                nc.vector.tensor_tensor(out=xt_s[:], in0=ps_xt[:],
                                        in1=bias_s['bxt'][:],
                                        op=ALU.add)
                xtT_s = pp.tile([128, 2, GPC], F16, tag="xtT")
                for i in range(2):
                    psT = p2t.tile([128, 128], F16, space="PSUM", tag="psT")
                    nc.tensor.transpose(out=psT[:, :GPC], in_=xt_s[:, 128 * i:128 * (i + 1)],
                                        identity=ident_s[:GPC, :GPC])
                    nc.vector.tensor_copy(out=xtT_s[:, i, :], in_=psT[:, :GPC])

            # ---------------- phase 3: GAT2 + GCN2 edge loop
            with tc.tile_pool(name="p3", bufs=3) as p3, \
                 tc.tile_pool(name="p3g", bufs=2) as p3g, \
                 tc.tile_pool(name="p3s", bufs=3, space="PSUM") as p3s, \
                 tc.tile_pool(name="p3acc", bufs=2, space="PSUM") as p3acc:
                for b in range(NBLK):
                    nt = TPBb[b]
                    ne = 128 * nt
                    v2g = p3g.tile([128, TPB, CROW], F16, tag="v2g")
                    nc.gpsimd.dma_gather(
                        out_ap=v2g[:, 0:nt, :], in_ap=comb_full[:],
                        idxs_ap=ixt2_s[:, b * (EPB // 16):b * (EPB // 16) + ne // 16],
                        num_idxs=ne, num_idxs_reg=ne, elem_size=CROW,
                        single_packet=False)
                    s01_b = p3g.tile([128, EPB], F16, tag="s01b3")
                    nc.sync.dma_start(out=s01_b[:], in_=s01_d[b])
                    s01t_b = p3g.tile([128, EPB], F16, tag="s01tb3")
                    nc.sync.dma_start(out=s01t_b[:], in_=s01t_d[b])
                    normb = p3g.tile([128, TPB], F32, tag="normb3")
                    nc.sync.dma_start(out=normb[:], in_=normv_d[b])
                    ps_agg = p3acc.tile([128, AGGW], F32, space="PSUM",
                                        tag="psagg3", name="psagg3")[:]
                    for k in range(nt):
                        s01_t = s01_b[:, 128 * k:128 * (k + 1)]
                        ps_l = p3s.tile([128, H], F32, space="PSUM", tag="psl3")
                        nc.tensor.matmul(out=ps_l[:], lhsT=s01t_b[:, 128 * k:128 * (k + 1)],
                                         rhs=adst2_s[:, b, :], start=True, stop=False)
                        nc.tensor.matmul(out=ps_l[:], lhsT=ident_s[:],
                                         rhs=v2g[:, k, HF:HF + H], start=False, stop=True)
                        lr02 = p3.tile([128, H], F32, tag="lr023")
                        nc.scalar.activation(out=lr02[:], in_=ps_l[:], func=AF.Copy, scale=0.2)
                        lr = p3.tile([128, H], F32, tag="lr3")
                        nc.vector.tensor_tensor(out=lr[:], in0=ps_l[:], in1=lr02[:], op=ALU.max)
                        ex_dup = p3.tile([128, H, 2], F16, tag="exdup3")
                        exv = p3.tile([128, AGGW], F16, tag="exv3")
                        nc.scalar.activation(out=exv[:, HF + F:AGGW], in_=lr[:],
                                             func=AF.Exp)
                        nc.vector.tensor_copy(
                            out=ex_dup[:],
                            in_=exv[:, HF + F:AGGW].rearrange(
                                "p (h one) -> p h one", one=1).to_broadcast([128, H, 2]))
                        nc.vector.tensor_tensor(
                            out=exv[:, 0:HF].rearrange("p (h f2 two) -> p h f2 two",
                                                       h=H, two=2),
                            in0=v2g[:, k, 0:HF].rearrange("p (h f2 two) -> p h f2 two",
                                                          h=H, two=2),
                            in1=ex_dup[:, :, None, :].to_broadcast([128, H, F // 2, 2]),
                            op=ALU.mult)
                        nc.vector.tensor_scalar(
                            out=exv[:, HF:HF + F], in0=v2g[:, k, HF + H:HF + H + F],
                            scalar1=normb[:, k:k + 1], scalar2=None, op0=ALU.mult)
                        nc.tensor.matmul(out=ps_agg[:, 0:512], lhsT=s01_t,
                                         rhs=exv[:, 0:512], start=(k == 0), stop=(k == nt - 1))
                        nc.tensor.matmul(out=ps_agg[:, 512:AGGW], lhsT=s01_t,
                                         rhs=exv[:, 512:AGGW], start=(k == 0), stop=(k == nt - 1))
                    rec = p3.tile([128, H], F32, tag="rec3")
                    nc.vector.reciprocal(out=rec[:], in_=ps_agg[:, HF + F:AGGW])
                    rec_dup = p3.tile([128, H, 2], F32, tag="recdup3")
                    nc.vector.tensor_copy(out=rec_dup[:],
                                          in_=rec[:, :, None].to_broadcast([128, H, 2]))
                    u_s = p3.tile([128, HF], F16, tag="us")
                    nc.vector.tensor_tensor(
                        out=u_s[:].rearrange("p (h f2 two) -> p h f2 two", h=H, two=2),
                        in0=ps_agg[:, 0:HF].rearrange("p (h f2 two) -> p h f2 two",
                                                      h=H, two=2),
                        in1=rec_dup[:, :, None, :].to_broadcast([128, H, F // 2, 2]),
                        op=ALU.mult)
                    # x1f = relu(u + bg2) -> persistent SBUF + DRAM (for pool gather)
                    v_s = p3.tile([128, HF], F16, tag="vs")
                    nc.vector.tensor_tensor(out=v_s[:], in0=u_s[:], in1=bg2_s[:], op=ALU.add)
                    nc.scalar.activation(out=x1f_s[:, b, :], in_=v_s[:], func=AF.Relu)
                    nc.sync.dma_start(out=x1f_dram[128 * b:128 * (b + 1), 0:HF],
                                      in_=x1f_s[:, b, :])
                    # x2f = relu(gcn_agg + bgcn)
                    g2f = p3.tile([128, F], F32, tag="g2f")
                    nc.vector.tensor_tensor(out=g2f[:], in0=ps_agg[:, HF:HF + F],
                                            in1=bgcnr_s[:], op=ALU.add)
                    nc.scalar.activation(out=x2f_s[:, b, :], in_=g2f[:], func=AF.Relu)
                    nc.sync.dma_start(out=x2f_dram[128 * b:128 * (b + 1), 0:F],
                                      in_=x2f_s[:, b, :])

            # ---------------- phase 4: pooling + head
            with tc.tile_pool(name="p4", bufs=2) as p4:
              with tc.tile_pool(name="p4s", bufs=1, space="PSUM") as p4s:
                gmax1T = pp.tile([128, 7, GPC], F16, tag="gmax1T")
                nc.vector.memset(gmax1T[:], 0.0)
                gmax2T = pp.tile([128, 1, GPC], F16, tag="gmax2T")
                nc.vector.memset(gmax2T[:], 0.0)
                CH = GPC // 2
                for h in range(2):
                    slab = p4.tile([128, 7, CH * PW], F16, tag="slab")
                    nc.gpsimd.dma_gather(
                        out_ap=slab[:], in_ap=x1f_dram[:],
                        idxs_ap=ixp_s[:, h * (CH * PW // 16):(h + 1) * (CH * PW // 16)],
                        num_idxs=CH * PW, num_idxs_reg=CH * PW, elem_size=CROW,
                        transpose=True, single_packet=False)
                    for j in range(7):
                        np_ = 128 if j < 6 else HF - 768   # skip NaN pad rows
                        nc.vector.tensor_reduce(
                            out=gmax1T[0:np_, j, h * CH:(h + 1) * CH].rearrange(
                                "p (g one) -> p g one", one=1),
                            in_=slab[0:np_, j, :].rearrange("p (g w) -> p g w", w=PW),
                            op=ALU.max, axis=AX)
                    slab2 = p4.tile([128, 1, CH * PW], F16, tag="slab2")
                    nc.gpsimd.dma_gather(
                        out_ap=slab2[:], in_ap=x2f_dram[:],
                        idxs_ap=ixp_s[:, h * (CH * PW // 16):(h + 1) * (CH * PW // 16)],
                        num_idxs=CH * PW, num_idxs_reg=CH * PW, elem_size=XROW,
                        transpose=True, single_packet=False)
                    nc.vector.tensor_reduce(
                        out=gmax2T[0:F, 0, h * CH:(h + 1) * CH].rearrange(
                            "p (g one) -> p g one", one=1),
                        in_=slab2[0:F, 0, :].rearrange("p (g w) -> p g w", w=PW),
                        op=ALU.max, axis=AX)
              # means via matmul, then transpose
              with tc.tile_pool(name="p4sm", bufs=1, space="PSUM") as p4s:
                ps_m1 = p4s.tile([GPC, HF], F32, space="PSUM", tag="psm1")
                ps_m2 = p4s.tile([GPC, F], F32, space="PSUM", tag="psm2")
                for b in range(NBLK):
                    nc.tensor.matmul(out=ps_m1[:, 0:512], lhsT=mmean_s[:, b, :],
                                     rhs=x1f_s[:, b, 0:512], start=(b == 0), stop=(b == NBLK - 1))
                    nc.tensor.matmul(out=ps_m1[:, 512:HF], lhsT=mmean_s[:, b, :],
                                     rhs=x1f_s[:, b, 512:HF], start=(b == 0), stop=(b == NBLK - 1))
                    nc.tensor.matmul(out=ps_m2[:], lhsT=mmean_s[:, b, :],
                                     rhs=x2f_s[:, b, :], start=(b == 0), stop=(b == NBLK - 1))
                mean1 = p4.tile([GPC, HF], F16, tag="mean1")
                nc.vector.tensor_copy(out=mean1[:], in_=ps_m1[:])
                mean2 = p4.tile([GPC, F], F16, tag="mean2")
                nc.vector.tensor_copy(out=mean2[:], in_=ps_m2[:])
              with tc.tile_pool(name="p4sh", bufs=1, space="PSUM") as p4s:
                gmean1T = pp.tile([128, 7, GPC], F16, tag="gmean1T")
                nc.vector.memset(gmean1T[:], 0.0)
                gmean2T = pp.tile([128, 1, GPC], F16, tag="gmean2T")
                nc.vector.memset(gmean2T[:], 0.0)
                for i in range(7):
                    c0, c1 = 128 * i, min(128 * (i + 1), HF)
                    psT = p4s.tile([128, 128], F16, space="PSUM", tag="psT4", bufs=2)
                    nc.tensor.transpose(out=psT[:c1 - c0, :GPC], in_=mean1[:, c0:c1],
                                        identity=ident_s[:GPC, :GPC])
                    nc.vector.tensor_copy(out=gmean1T[0:c1 - c0, i, :], in_=psT[:c1 - c0, :GPC])
                psT = p4s.tile([128, 128], F16, space="PSUM", tag="psT4", bufs=2)
                nc.tensor.transpose(out=psT[:F, :GPC], in_=mean2[:], identity=ident_s[:GPC, :GPC])
                nc.vector.tensor_copy(out=gmean2T[0:F, 0, :], in_=psT[:F, :GPC])

                def head_mm(ps, chunks, rhs_tile, nw):
                    n = len(chunks)
                    for i, ch in enumerate(chunks):
                        nc.tensor.matmul(out=ps[:], lhsT=ch, rhs=rhs_tile[:, i, :nw],
                                         start=(i == 0), stop=(i == n - 1))

                def bias_relu_T(ps, bias_ap, w, relu, nT, tagb):
                    zs = p4.tile([GPC, w], F16, tag="z" + tagb)
                    nc.vector.tensor_tensor(out=zs[:], in0=ps[:],
                                            in1=bias_ap, op=ALU.add)
                    if relu:
                        nc.vector.tensor_scalar(out=zs[:], in0=zs[:], scalar1=0.0,
                                                scalar2=None, op0=ALU.max)
                    zT = pp.tile([128, nT, GPC], F16, tag="zT" + tagb)
                    for i in range(nT):
                        psT2 = p4s.tile([128, 128], F16, space="PSUM", tag="psT4", bufs=2)
                        nc.tensor.transpose(out=psT2[:, :GPC], in_=zs[:, 128 * i:128 * (i + 1)],
                                            identity=ident_s[:GPC, :GPC])
                        nc.vector.tensor_copy(out=zT[:, i, :], in_=psT2[:, :GPC])
                    return zT

                ps_z1 = p4s.tile([GPC, 128], F32, space="PSUM", tag="psz1")
                head_mm(ps_z1, [gmax1T[:, j, :] for j in range(7)]
                        + [gmean1T[:, j, :] for j in range(7)], wfg1_s, 128)
                z1T = bias_relu_T(ps_z1, bias_s['bfg1'][:], 128, True, 1, "1")
                ps_z2 = p4s.tile([GPC, 128], F32, space="PSUM", tag="psz2")
                head_mm(ps_z2, [gmax2T[:, 0, :], gmean2T[:, 0, :]], wfg2_s, 128)
                z2T = bias_relu_T(ps_z2, bias_s['bfg2'][:], 128, True, 1, "2")
                ps_h1 = p4s.tile([GPC, 512], F32, space="PSUM", tag="psh1")
                head_mm(ps_h1, [z1T[:, 0, :], z2T[:, 0, :], xtT_s[:, 0, :], xtT_s[:, 1, :]],
                        w1_s, 512)
                h1T = bias_relu_T(ps_h1, bias_s['b1'][:], 512, True, 4, "h1")
                ps_h2 = p4s.tile([GPC, 256], F32, space="PSUM", tag="psh2")
                head_mm(ps_h2, [h1T[:, i, :] for i in range(4)], w2_s, 256)
                h2T = bias_relu_T(ps_h2, bias_s['b2'][:], 256, True, 2, "h2")
                ps_o = p4s.tile([GPC, 1], F32, space="PSUM", tag="pso")
                head_mm(ps_o, [h2T[:, i, :] for i in range(2)], wo_s, 1)
                o_s = p4.tile([GPC, 1], F32, tag="os")
                nc.vector.tensor_scalar(out=o_s[:], in0=ps_o[:], scalar1=bo_s[:],
                                        scalar2=None, op0=ALU.add)
                nc.sync.dma_start(out=out_d[:], in_=o_s[:])

    nc.compile()
    return nc


def build_in_maps(nc, shared, cores):
    declared = set()
    import concourse.mybir as _mb
    for alloc in nc.m.functions[0].allocations:
        if isinstance(alloc, _mb.MemoryLocationSet) and alloc.kind == "ExternalInput":
            declared.add(alloc.memorylocations[0].name)
    in_maps = []
    for c in range(8):
        m = dict(shared)
        m.update(cores[c])
        in_maps.append({k: np.ascontiguousarray(v) for k, v in m.items()
                        if k in declared})
    return in_maps


_CACHE = {}


def run_device(inputs):
    meta, shared, cores = prep(**inputs)
    key = (meta['NBLK'], meta['TPB'], meta['PW'], meta['TPBb'])
    if key not in _CACHE:
        _CACHE[key] = build(meta)
    nc = _CACHE[key]
    in_maps = build_in_maps(nc, shared, cores)
    res = run_bass_kernel_spmd(nc, in_maps, core_ids=list(range(8)))
    out = np.concatenate([res.results[c]['out'] for c in range(8)], axis=0)
    return out.astype(np.float32)


def kernel(**inputs):
    return run_device(inputs)


# revision 4
# speedup vs baseline: 1.1112x; 1.0416x over previous
"""Trainium2 Bass kernel for nn_GAT_GCN (gnn_message_passing), 8 NeuronCores.

v2 strategy (restructured from baseline):
 - Projection commutes with the per-head softmax aggregation, so both GAT
   layers aggregate RAW per-edge features (x for layer 1, h2 for layer 2) and
   the projections are folded on the host: W2eff = blockdiag(Wg1) @ Wg2 etc.
   Phase 1 therefore needs no per-edge projection matmuls at all.
 - Per-edge alpha scaling uses a duplicated-exponent layout (ex_dup[128,H,2])
   so every TensorTensor operand is packed f16 SBUF -> DVE 2x mode.
 - leaky_relu(x) = 0.6x + 0.4|x| is fused into two Activation-engine ops.
 - One merged node table [h2 | asrc2 | xp] (896 cols) is exchanged with a
   single AllGather and gathered per edge once in phase 3.
 - Segment softmax/scatter-add remain matmuls against host-built 0/1
   selection blocks; GCN norm is applied per-edge via tensor_scalar.
"""
import sys
sys.path.insert(0, '/opt/trn_rl_repo')
import numpy as np

N, E, G, F, H = 16384, 131072, 128, 78, 10
NCORE, GPC = 8, 16          # cores, graphs per core
HF = H * F                  # 780
W2AUG = HF + 2 * H          # 800 = h2 | asrc2 | adst2
XROW = 128                  # x2f table row, f16 (256B)
XAROW = 256                 # xa table row, f16 (512B): [x 78 | asrc1 10 | pad | xd 78]
CROW = 896                  # comb/x1f row, f16 (1792B): [h2 780 | asrc2 10 | xp 78 | pad]
AGGW = HF + F + H           # 868 = gat 780 | gcn 78 | den 10


def _wrap16(v):
    """dma_gather idx layout: [128, len/16] int16, idx i at (i%16, i//16),
    replicated across the 8 Q7 core groups."""
    v = np.asarray(v, np.int16)
    assert len(v) % 16 == 0
    m = v.reshape(-1, 16).T            # [16, S]
    return np.tile(m, (8, 1)).copy()   # [128, S]


def _f16(a):
    return np.ascontiguousarray(np.asarray(a, np.float32)).astype(np.float16)


def prep(x, edge_index, batch, target, Wg1, as1, ad1, bg1, Wg2, as2, ad2, bg2,
         Wgcn, bgcn, Wfg1, bfg1, Wfg2, bfg2, wconv, bconv, Wxt, bxt,
         W1, b1, W2, b2, Wo, bo):
    x = np.asarray(x, np.float64)
    ei = np.asarray(edge_index, np.int64)
    batch = np.asarray(batch, np.int64)
    target = np.asarray(target, np.float32)
    Wg1 = np.asarray(Wg1, np.float64); Wg2 = np.asarray(Wg2, np.float64)
    as1 = np.asarray(as1, np.float64); ad1 = np.asarray(ad1, np.float64)
    as2 = np.asarray(as2, np.float64); ad2 = np.asarray(ad2, np.float64)
    bg1 = np.asarray(bg1, np.float64)
    Wgcn = np.asarray(Wgcn, np.float64); bgcn_ = np.asarray(bgcn, np.float64)

    loops = np.arange(N, dtype=np.int64)
    src = np.concatenate([ei[0], loops])
    dst = np.concatenate([ei[1], loops])

    # ---- host-folded weights
    B_s1 = np.zeros((F, H)); B_d1 = np.zeros((F, H))
    for h in range(H):
        B_s1[:, h] = Wg1[:, h * F:(h + 1) * F] @ as1[h]
        B_d1[:, h] = Wg1[:, h * F:(h + 1) * F] @ ad1[h]
    asrc1 = x @ B_s1                                  # [N, H]
    adst1 = x @ B_d1                                  # [N, H]
    Wt1 = np.zeros((HF, HF))
    for h in range(H):
        Wt1[h * F:(h + 1) * F, h * F:(h + 1) * F] = Wg1[:, h * F:(h + 1) * F]
    B_s2 = (Wg2.reshape(HF, H, F) * as2[:, None, :].transpose(1, 0, 2)).sum(-1)
    B_d2 = (Wg2.reshape(HF, H, F) * ad2[:, None, :].transpose(1, 0, 2)).sum(-1)
    Wcat = np.concatenate([Wg2, B_s2, B_d2], axis=1)  # [780, 800]
    W2effaug = Wt1 @ Wcat                             # [780, 800]
    c2aug = bg1 @ Wcat                                # [800]
    Wgcn2 = Wgcn @ Wgcn
    cgcn = bgcn_ @ Wgcn

    # graph-aligned core boundaries
    counts = np.bincount(batch, minlength=G)
    node_off = np.concatenate([[0], np.cumsum(counts)])
    n_lo = node_off[np.arange(NCORE) * GPC]
    n_hi = node_off[(np.arange(NCORE) + 1) * GPC]

    # degrees / gcn norm (over full edge list incl self loops)
    deg = np.bincount(dst, minlength=N).astype(np.float64)
    dinv = 1.0 / np.sqrt(deg)
    norm = (dinv[src] * dinv[dst]).astype(np.float32)

    order = np.argsort(dst, kind='stable')
    srcs, dsts, norms = src[order], dst[order], norm[order]

    Lmax = int((n_hi - n_lo).max())
    NBLK = (Lmax + 127) // 128
    NPC = NBLK * 128
    assert NCORE * NPC < 32768

    node_owner = np.searchsorted(n_hi - 1, np.arange(N), side='left')
    node_owner = np.minimum(node_owner, NCORE - 1)
    pad_gid = node_owner * NPC + (np.arange(N) - n_lo[node_owner])

    # per (core, block) edge spans -> uniform TPB
    spans = []
    TPB = 1
    for c in range(NCORE):
        e1 = np.searchsorted(dsts, n_hi[c])
        bl = []
        for b in range(NBLK):
            lo = np.searchsorted(dsts, n_lo[c] + 128 * b)
            hi = np.searchsorted(dsts, min(n_lo[c] + 128 * (b + 1), n_hi[c]))
            if n_lo[c] + 128 * b >= n_hi[c]:
                lo = hi = e1
            bl.append((lo, hi))
            TPB = max(TPB, (hi - lo + 127) // 128)
        spans.append(bl)
    ET = NBLK * TPB
    ECAP = ET * 128
    TPBb = []
    for b in range(NBLK):
        m = 1
        for c in range(NCORE):
            lo, hi = spans[c][b]
            m = max(m, (hi - lo + 127) // 128)
        TPBb.append(int(m))

    PW = int(np.ceil(counts.max() / 16) * 16)   # pool slot width per graph

    cores = []
    for c in range(NCORE):
        esrc = np.zeros(ECAP, np.int64)           # raw src id per edge slot
        s01 = np.zeros((NBLK, 128, TPB * 128), np.float16)
        s01t = np.zeros((NBLK, 128, TPB * 128), np.float16)
        for b in range(NBLK):
            lo, hi = spans[c][b]
            ne = hi - lo
            if ne == 0:
                continue
            sl = slice(b * TPB * 128, b * TPB * 128 + ne)
            esrc[sl] = srcs[lo:hi]
            ld = (dsts[lo:hi] - n_lo[c] - 128 * b).astype(np.int64)  # 0..127
            j = np.arange(ne)
            t_loc = j // 128
            e_loc = j % 128
            s01[b, e_loc, t_loc * 128 + ld] = 1.0
            s01t[b, ld, j] = 1.0
        # padded dst columns (no incoming edges) get one dummy S entry so the
        # softmax denominator stays finite (their rows are garbage, never read)
        Lc = int(n_hi[c] - n_lo[c])
        for b in range(NBLK):
            first_pad = max(0, min(128, Lc - 128 * b))
            if first_pad < 128:
                s01[b, 0, first_pad:128] = 1.0

        # adst1 / dinv for local dst nodes
        adst1loc = np.zeros((NBLK, 128, H), np.float16)
        av = adst1[n_lo[c]:n_hi[c]].astype(np.float16)
        ids = np.arange(Lc)
        adst1loc[ids // 128, ids % 128, :] = av
        dinvloc = np.ones((128, NBLK), np.float32)
        dinvloc[ids % 128, ids // 128] = dinv[n_lo[c]:n_hi[c]].astype(np.float32)

        # pooling indices (local node ids into x1f/x2f tables)
        pool_idx = np.zeros(GPC * PW, np.int64)
        for g in range(GPC):
            gg = c * GPC + g
            a, bnd = node_off[gg] - n_lo[c], node_off[gg + 1] - n_lo[c]
            cnt = bnd - a
            pool_idx[g * PW:g * PW + cnt] = np.arange(a, bnd)
            pool_idx[g * PW + cnt:(g + 1) * PW] = a      # pad = first node of graph
        mmean = np.zeros((NBLK, 128, GPC), np.float16)
        for g in range(GPC):
            gg = c * GPC + g
            a, bnd = node_off[gg] - n_lo[c], node_off[gg + 1] - n_lo[c]
            ids = np.arange(a, bnd)
            mmean[ids // 128, ids % 128, g] = np.float16(1.0 / (bnd - a))

        # conv im2col: [32, GPC, 608]
        t_win = np.zeros((32, GPC, 608), np.float16)
        tg = target[c * GPC:(c + 1) * GPC, 0, :]          # [GPC, 625]
        for k in range(32):
            t_win[k, :, :594] = tg[:, k:k + 594].astype(np.float16)

        cores.append(dict(
            ix_x=_wrap16(esrc),                     # for xa-gather (raw ids)
            ix_t2=_wrap16(pad_gid[esrc]),           # for comb gathers (padded ids)
            ix_pool=_wrap16(pool_idx),
            s01=s01, s01t=s01t, dinvloc=dinvloc,
            adst1loc=np.ascontiguousarray(adst1loc.transpose(1, 0, 2)),
            mmean=np.ascontiguousarray(mmean.transpose(1, 0, 2)), t_win=t_win,
            bconv_rep=np.full((GPC, 1), float(np.asarray(bconv).reshape(-1)[0]), np.float32),
        ))

    # gather table: [x | asrc1 | pad | x*dinv]
    xa16 = np.zeros((N, XAROW), np.float16)
    xa16[:, :F] = x.astype(np.float16)
    xa16[:, F:F + H] = asrc1.astype(np.float16)
    xa16[:, 128:128 + F] = (x * dinv[:, None]).astype(np.float16)

    # W2effaug split into 7 chunks of 128 rows over a [896, 800] matrix whose
    # rows 780:896 are zero (transposed aggregate cols 780+ are gcn/garbage)
    w2eff = np.zeros((7, 128, W2AUG), np.float16)
    for k in range(7):
        r0, r1 = 128 * k, min(128 * (k + 1), HF)
        w2eff[k, :r1 - r0, :] = W2effaug[r0:r1].astype(np.float16)

    wgcn2_s = np.zeros((128, F), np.float16)
    wgcn2_s[:F] = Wgcn2.astype(np.float16)

    def pack_rows(Wm, splits, ncol):
        out = np.zeros((len(splits), 128, ncol), np.float16)
        for i, (r0, r1) in enumerate(splits):
            out[i, :r1 - r0, :] = _f16(Wm[r0:r1, :])
        return out

    sp7 = [(128 * i, min(128 * (i + 1), HF)) for i in range(7)]
    wfg1p = np.concatenate([pack_rows(np.asarray(Wfg1)[:HF], sp7, 128),
                            pack_rows(np.asarray(Wfg1)[HF:], sp7, 128)], axis=0)
    wfg2p = pack_rows(np.asarray(Wfg2), [(0, F), (F, 2 * F)], 128)
    wxtp = pack_rows(np.asarray(Wxt), [(128 * i, min(128 * (i + 1), 594)) for i in range(5)], 256)
    w1p = pack_rows(np.asarray(W1), [(128 * i, 128 * (i + 1)) for i in range(4)], 512)
    w2p = pack_rows(np.asarray(W2), [(128 * i, 128 * (i + 1)) for i in range(4)], 256)
    wop = pack_rows(np.asarray(Wo), [(0, 128), (128, 256)], 1)

    shared = dict(
        xa16=xa16, w2eff=np.ascontiguousarray(w2eff.transpose(1, 0, 2)),
        c2row=c2aug.astype(np.float16).reshape(1, W2AUG),
        wgcn2_s=wgcn2_s,
        cgcn_row=cgcn.astype(np.float32).reshape(1, F),
        bgcn_row=np.asarray(bgcn, np.float32).reshape(1, F),
        bg2row=np.asarray(bg2, np.float16).reshape(1, HF),
        wfg1p=np.ascontiguousarray(wfg1p.transpose(1, 0, 2)),
        bfg1=np.asarray(bfg1, np.float32).reshape(1, 128),
        wfg2p=np.ascontiguousarray(wfg2p.transpose(1, 0, 2)),
        bfg2=np.asarray(bfg2, np.float32).reshape(1, 128),
        wxtp=np.ascontiguousarray(wxtp.transpose(1, 0, 2)),
        bxt=np.asarray(bxt, np.float32).reshape(1, 256),
        w1p=np.ascontiguousarray(w1p.transpose(1, 0, 2)),
        b1=np.asarray(b1, np.float32).reshape(1, 512),
        w2p=np.ascontiguousarray(w2p.transpose(1, 0, 2)),
        b2=np.asarray(b2, np.float32).reshape(1, 256),
        wop=np.ascontiguousarray(wop.transpose(1, 0, 2)), bo_rep=np.full((GPC, 1), float(np.asarray(bo).reshape(-1)[0]), np.float32),
        w_sel=np.zeros((32, GPC, GPC), np.float16),
    )
    wcol = _f16(np.asarray(wconv).reshape(-1))
    for g in range(GPC):
        shared['w_sel'][:, g, g] = wcol

    BH = 1
    for c in range(NCORE):
        half_end = node_off[c * GPC + GPC // 2] - n_lo[c]
        BH = max(BH, int(-(-half_end // 128)))
    meta = dict(NBLK=NBLK, NPC=NPC, TPB=TPB, ET=ET, ECAP=ECAP, PW=PW,
                TPBb=tuple(TPBb), BH=BH, n_lo=n_lo, n_hi=n_hi)
    return meta, shared, cores


import concourse.bass as bass
import concourse.bacc as bacc
import concourse.mybir as mybir
from concourse import library_config
from concourse.tile import TileContext
from concourse.masks import make_identity
from concourse.bass_utils import run_bass_kernel_spmd

F16 = mybir.dt.float16
F32 = mybir.dt.float32
I16 = mybir.dt.int16
AX = mybir.AxisListType.X
ALU = mybir.AluOpType
AF = mybir.ActivationFunctionType


def build(meta):
    NBLK, NPC, TPB, ET, ECAP, PW = (meta[k] for k in
                                    ['NBLK', 'NPC', 'TPB', 'ET', 'ECAP', 'PW'])
    TPBb = meta['TPBb']
    BH = meta['BH']
    EPB = TPB * 128                       # edges per block
    nc = bacc.Bacc()

    dp = lambda n, s, d: nc.declare_dram_parameter(n, list(s), d, isOutput=False)
    # per-core inputs
    xa16 = dp('xa16', [N, XAROW], F16)
    ix_x = dp('ix_x', [128, ECAP // 16], I16)
    ix_t2 = dp('ix_t2', [128, ECAP // 16], I16)
    ix_pool = dp('ix_pool', [128, GPC * PW // 16], I16)
    s01_d = dp('s01', [NBLK, 128, EPB], F16)
    s01t_d = dp('s01t', [NBLK, 128, EPB], F16)
    dinv_d = dp('dinvloc', [128, NBLK], F32)
    adst1_d = dp('adst1loc', [128, NBLK, H], F16)
    mmean_d = dp('mmean', [128, NBLK, GPC], F16)
    twin_d = dp('t_win', [32, GPC, 608], F16)
    bconv_rep = dp('bconv_rep', [GPC, 1], F32)
    # shared weights
    w2eff_d = dp('w2eff', [128, 7, W2AUG], F16)
    c2row = dp('c2row', [1, W2AUG], F16)
    wgcn2 = dp('wgcn2_s', [128, F], F16)
    cgcn_row = dp('cgcn_row', [1, F], F32)
    bgcn_row = dp('bgcn_row', [1, F], F32)
    bg2row = dp('bg2row', [1, HF], F16)
    wfg1p = dp('wfg1p', [128, 14, 128], F16)
    bfg1 = dp('bfg1', [1, 128], F32)
    wfg2p = dp('wfg2p', [128, 2, 128], F16)
    bfg2 = dp('bfg2', [1, 128], F32)
    wxtp = dp('wxtp', [128, 5, 256], F16)
    bxt = dp('bxt', [1, 256], F32)
    w1p = dp('w1p', [128, 4, 512], F16)
    b1 = dp('b1', [1, 512], F32)
    w2p = dp('w2p', [128, 4, 256], F16)
    b2 = dp('b2', [1, 256], F32)
    wop = dp('wop', [128, 2, 1], F16)
    bo_rep = dp('bo_rep', [GPC, 1], F32)
    wsel_d = dp('w_sel', [32, GPC, GPC], F16)

    out_d = nc.declare_dram_parameter('out', [GPC, 1], F32, isOutput=True)

    # internal DRAM
    comb_shard = nc.dram_tensor('comb_shard', [NPC, CROW], F16)
    comb_full = nc.dram_tensor('comb_full', [8 * NPC, CROW], F16, addr_space="Shared")
    x1f_dram = nc.dram_tensor('x1f_dram', [NPC, CROW], F16)
    x2f_dram = nc.dram_tensor('x2f_dram', [NPC, XROW], F16)

    RG = [list(range(8))]

    with TileContext(nc) as tc:
        nc.gpsimd.load_library(library_config.mlp)

        with tc.tile_pool(name="persist", bufs=1) as pp:
            # ---------------- persistent tiles + loads
            adst1_s = pp.tile([128, NBLK, H], F16, tag="adst1")
            nc.sync.dma_start(out=adst1_s[:], in_=adst1_d[:])
            dinv_s = pp.tile([128, NBLK], F32, tag="dinv")
            nc.sync.dma_start(out=dinv_s[:], in_=dinv_d[:])
            w2eff_s = pp.tile([128, 7, W2AUG], F16, tag="w2eff")
            nc.sync.dma_start(out=w2eff_s[:], in_=w2eff_d[:])
            c2_s = pp.tile([1, W2AUG], F16, tag="c2")
            nc.sync.dma_start(out=c2_s[:], in_=c2row[:])
            wgcn2_s = pp.tile([128, F], F16, tag="wgcn2")
            nc.sync.dma_start(out=wgcn2_s[:], in_=wgcn2[:])
            cgcn_s = pp.tile([128, F], F32, tag="cgcn")
            nc.sync.dma_start(out=cgcn_s[:], in_=cgcn_row[:].to_broadcast([128, F]))
            bgcnr_s = pp.tile([128, F], F32, tag="bgcnr")
            nc.sync.dma_start(out=bgcnr_s[:], in_=bgcn_row[:].to_broadcast([128, F]))
            bg2_s = pp.tile([128, HF], F16, tag="bg2")
            nc.sync.dma_start(out=bg2_s[:], in_=bg2row[:].to_broadcast([128, HF]))
            ixx_s = pp.tile([128, ECAP // 16], I16, tag="ixx")
            nc.sync.dma_start(out=ixx_s[:], in_=ix_x[:])
            ixt2_s = pp.tile([128, ECAP // 16], I16, tag="ixt2")
            nc.sync.dma_start(out=ixt2_s[:], in_=ix_t2[:])
            ixp_s = pp.tile([128, GPC * PW // 16], I16, tag="ixp")
            nc.sync.dma_start(out=ixp_s[:], in_=ix_pool[:])
            mmean_s = pp.tile([128, NBLK, GPC], F16, tag="mmean")
            nc.sync.dma_start(out=mmean_s[:], in_=mmean_d[:])
            wsel_s = pp.tile([32, GPC, GPC], F16, tag="wsel")
            nc.sync.dma_start(out=wsel_s[:], in_=wsel_d[:])
            bconv_s = pp.tile([GPC, 1], F32, tag="bconv")
            nc.sync.dma_start(out=bconv_s[:], in_=bconv_rep[:])
            wfg1_s = pp.tile([128, 14, 128], F16, tag="wfg1")
            nc.sync.dma_start(out=wfg1_s[:], in_=wfg1p[:])
            wfg2_s = pp.tile([128, 2, 128], F16, tag="wfg2")
            nc.sync.dma_start(out=wfg2_s[:], in_=wfg2p[:])
            wxt_s = pp.tile([128, 5, 256], F16, tag="wxt")
            nc.sync.dma_start(out=wxt_s[:], in_=wxtp[:])
            w1_s = pp.tile([128, 4, 512], F16, tag="w1")
            nc.sync.dma_start(out=w1_s[:], in_=w1p[:])
            w2_s = pp.tile([128, 4, 256], F16, tag="w2")
            nc.sync.dma_start(out=w2_s[:], in_=w2p[:])
            wo_s = pp.tile([128, 2, 1], F16, tag="wo")
            nc.sync.dma_start(out=wo_s[:], in_=wop[:])
            bias_s = {}
            for nm, t, w in [('bfg1', bfg1, 128), ('bfg2', bfg2, 128),
                             ('bxt', bxt, 256), ('b1', b1, 512), ('b2', b2, 256)]:
                bias_s[nm] = pp.tile([GPC, w], F32, tag="bias_" + nm, name="bias_" + nm)
                nc.sync.dma_start(out=bias_s[nm][:], in_=t[:].to_broadcast([GPC, w]))
            bo_s = pp.tile([GPC, 1], F32, tag="bo")
            nc.sync.dma_start(out=bo_s[:], in_=bo_rep[:])

            ident_s = pp.tile([128, 128], F16, tag="ident")
            make_identity(nc, ident_s[:])
            ones_s = pp.tile([1, 128], F16, tag="ones")
            nc.vector.memset(ones_s[:], 1.0)

            # work state
            adst2_s = pp.tile([128, NBLK, H], F16, tag="adst2")
            x1loc_s = pp.tile([128, NBLK, CROW], F16, tag="x1loc")
            nc.vector.memset(x1loc_s[:, :, HF + F:CROW], 0.0)
            t2stage = pp.tile([128, CROW], F16, tag="t2stage")
            nc.vector.memset(t2stage[:], 0.0)
            x1f_s = pp.tile([128, NBLK, HF], F16, tag="x1fs")
            x2f_s = pp.tile([128, NBLK, F], F16, tag="x2fs")
            gmax1T = pp.tile([128, 7, GPC], F16, tag="gmax1T")
            nc.vector.memset(gmax1T[:], 0.0)
            gmax2T = pp.tile([128, 1, GPC], F16, tag="gmax2T")
            nc.vector.memset(gmax2T[:], 0.0)
            CH = GPC // 2
            # zero-fill the pad columns of the pooled-row tables once (the
            # pool gather reads whole rows; uninit DRAM is NaN in the sim)
            zpad = pp.tile([128, NBLK, CROW - HF], F16, tag="zpad")
            nc.vector.memset(zpad[:], 0.0)
            nc.sync.dma_start(
                out=x1f_dram[:].rearrange("(b p) c -> p b c", p=128)[:, :, HF:CROW],
                in_=zpad[:])
            nc.sync.dma_start(
                out=x2f_dram[:].rearrange("(b p) c -> p b c", p=128)[:, :, F:XROW],
                in_=zpad[:, :, 0:XROW - F])

            def edge_tile(p, ps, xg, s01_b, s01t_b, normb, adst_ap, b, k, src_off):
                """Per-edge-tile softmax + aggregation. xg rows are
                [payload | asrc (H at src_off) | ...]; payload cols 0:780 for
                GAT (broadcast over heads in phase 1), gcn payload at gcn_off.
                Returns nothing; accumulates into ps_agg (caller-held)."""
                pass  # logic inlined below per phase (layouts differ)

            # ---------------- phase 1: GAT1 + GCN1 edge loop (raw-x aggregation)
            with tc.tile_pool(name="p1", bufs=3) as p1, \
                 tc.tile_pool(name="p1g", bufs=2) as p1g, \
                 tc.tile_pool(name="p1s", bufs=3, space="PSUM") as p1s, \
                 tc.tile_pool(name="p1acc", bufs=2, space="PSUM") as p1acc:
                for b in range(NBLK):
                    nt = TPBb[b]
                    ne = 128 * nt
                    xg = p1g.tile([128, TPB, XAROW], F16, tag="xg")
                    nc.gpsimd.dma_gather(
                        out_ap=xg[:, 0:nt, :], in_ap=xa16[:],
                        idxs_ap=ixx_s[:, b * (EPB // 16):b * (EPB // 16) + ne // 16],
                        num_idxs=ne, num_idxs_reg=ne, elem_size=XAROW,
                        single_packet=False)
                    s01_b = p1g.tile([128, EPB], F16, tag="s01b")
                    nc.sync.dma_start(out=s01_b[:], in_=s01_d[b])
                    s01t_b = p1g.tile([128, EPB], F16, tag="s01tb")
                    nc.sync.dma_start(out=s01t_b[:], in_=s01t_d[b])
                    ps_agg = p1acc.tile([128, AGGW], F32, space="PSUM",
                                        tag="psagg", name="psagg")[:]
                    for k in range(nt):
                        s01_t = s01_b[:, 128 * k:128 * (k + 1)]
                        ps_l = p1s.tile([128, H], F32, space="PSUM", tag="psl")
                        nc.tensor.matmul(out=ps_l[:], lhsT=s01t_b[:, 128 * k:128 * (k + 1)],
                                         rhs=adst1_s[:, b, :], start=True, stop=False)
                        nc.tensor.matmul(out=ps_l[:], lhsT=ident_s[:],
                                         rhs=xg[:, k, F:F + H], start=False, stop=True)
                        # exp(leaky_relu(lg)), lg = max(lg, 0.2*lg)
                        # exp(leaky(x)) = max(exp(x), exp(0.2x))
                        e1 = p1.tile([128, H], F16, tag="e1")
                        nc.scalar.activation(out=e1[:], in_=ps_l[:], func=AF.Exp)
                        e2 = p1.tile([128, H], F16, tag="e2")
                        nc.scalar.activation(out=e2[:], in_=ps_l[:], func=AF.Exp, scale=0.2)
                        ex_dup = p1.tile([128, H, 2], F16, tag="exdup")
                        exv = p1.tile([128, AGGW], F16, tag="exv")
                        nc.vector.tensor_tensor(out=exv[:, HF + F:AGGW], in0=e1[:],
                                                in1=e2[:], op=ALU.max)
                        nc.vector.tensor_copy(
                            out=ex_dup[:],
                            in_=exv[:, HF + F:AGGW].rearrange(
                                "p (h one) -> p h one", one=1).to_broadcast([128, H, 2]))
                        # exv[:, 0:780] = x broadcast over heads * ex
                        nc.vector.tensor_tensor(
                            out=exv[:, 0:HF].rearrange("p (h f2 two) -> p h f2 two",
                                                       h=H, two=2),
                            in0=xg[:, k, 0:F].rearrange("p (f2 two) -> p f2 two", two=2)
                                [:, None, :, :].to_broadcast([128, H, F // 2, 2]),
                            in1=ex_dup[:, :, None, :].to_broadcast([128, H, F // 2, 2]),
                            op=ALU.mult)
                        # exv[:, 780:858] = x * norm (per-partition scalar)
                        nc.tensor.matmul(out=ps_agg[:, 0:512], lhsT=s01_t,
                                         rhs=exv[:, 0:512], start=(k == 0), stop=(k == nt - 1))
                        nc.tensor.matmul(out=ps_agg[:, 512:HF], lhsT=s01_t,
                                         rhs=exv[:, 512:HF], start=(k == 0), stop=False)
                        nc.tensor.matmul(out=ps_agg[:, HF:HF + F], lhsT=s01_t,
                                         rhs=xg[:, k, 128:128 + F], start=False, stop=False)
                        nc.tensor.matmul(out=ps_agg[:, HF + F:AGGW], lhsT=s01_t,
                                         rhs=exv[:, HF + F:AGGW], start=False, stop=(k == nt - 1))
                    # normalize gat part per head; copy gcn part
                    rec = p1.tile([128, H], F32, tag="rec")
                    nc.vector.reciprocal(out=rec[:], in_=ps_agg[:, HF + F:AGGW])
                    rec_dup = p1.tile([128, H, 2], F16, tag="recdup")
                    nc.vector.tensor_copy(out=rec_dup[:],
                                          in_=rec[:, :, None].to_broadcast([128, H, 2]))
                    aggf = p1.tile([128, HF], F16, tag="aggf")
                    nc.scalar.activation(out=aggf[:], in_=ps_agg[:, 0:HF], func=AF.Copy)
                    nc.vector.tensor_tensor(
                        out=x1loc_s[:, b, 0:HF].rearrange("p (h f2 two) -> p h f2 two",
                                                          h=H, two=2),
                        in0=aggf[:].rearrange("p (h f2 two) -> p h f2 two",
                                              h=H, two=2),
                        in1=rec_dup[:, :, None, :].to_broadcast([128, H, F // 2, 2]),
                        op=ALU.mult)
                    nc.scalar.activation(out=x1loc_s[:, b, HF:HF + F],
                                         in_=ps_agg[:, HF:HF + F], func=AF.Copy,
                                         scale=dinv_s[:, b:b + 1])

            # ---------------- phase 2: T2 table (folded proj), collective, conv
            with tc.tile_pool(name="p2", bufs=2) as p2:
              with tc.tile_pool(name="p2sb", bufs=2, space="PSUM") as p2s, \
                   tc.tile_pool(name="p2tb", bufs=2, space="PSUM") as p2t:
                # aggn transposes -> x1t_s  [chunk r, 7, node]; gcn agg gets its
                # own partition-0-aligned transpose
                x1t_s = p2.tile([128, 7, NPC], F16, tag="x1t", bufs=1)
                gcnT_s = p2.tile([128, NPC], F16, tag="gcnT", bufs=1)
                for b in range(NBLK):
                    for fb in range(7):
                        psT = p2t.tile([128, 128], F16, space="PSUM", tag="psT")
                        nc.tensor.transpose(out=psT[:],
                                            in_=x1loc_s[:, b, 128 * fb:128 * (fb + 1)],
                                            identity=ident_s[:])
                        nc.vector.tensor_copy(
                            out=x1t_s[:, fb, 128 * b:128 * (b + 1)],
                            in_=psT[:])
                    psT = p2t.tile([128, 128], F16, space="PSUM", tag="psT")
                    nc.tensor.transpose(out=psT[:F, :],
                                        in_=x1loc_s[:, b, HF:HF + F],
                                        identity=ident_s[:])
                    nc.vector.tensor_copy(out=gcnT_s[0:F, 128 * b:128 * (b + 1)],
                                          in_=psT[:F, :])
                # T2 build + xp (gcn) per block
                for b in range(NBLK):
                    ps_t2 = p2s.tile([128, W2AUG], F32, space="PSUM", tag="pst2")
                    for k in range(7):
                        nc.tensor.matmul(out=ps_t2[:, 0:512],
                                         lhsT=x1t_s[:, k, 128 * b:128 * (b + 1)],
                                         rhs=w2eff_s[:, k, 0:512], start=(k == 0), stop=False)
                        nc.tensor.matmul(out=ps_t2[:, 512:W2AUG],
                                         lhsT=x1t_s[:, k, 128 * b:128 * (b + 1)],
                                         rhs=w2eff_s[:, k, 512:W2AUG], start=(k == 0), stop=False)
                    nc.tensor.matmul(out=ps_t2[:, 0:512], lhsT=ones_s[:],
                                     rhs=c2_s[:, 0:512], start=False, stop=True)
                    nc.tensor.matmul(out=ps_t2[:, 512:W2AUG], lhsT=ones_s[:],
                                     rhs=c2_s[:, 512:W2AUG], start=False, stop=True)
                    # xp = agg_gcn @ Wgcn^2 + cgcn ; agg_gcn rows are chunk-6
                    # local rows 12:90 of the transposed aggregate
                    ps_xp = p2s.tile([128, F], F32, space="PSUM", tag="psxp")
                    nc.tensor.matmul(out=ps_xp[:],
                                     lhsT=gcnT_s[0:F, 128 * b:128 * (b + 1)],
                                     rhs=wgcn2_s[0:F, :], start=True, stop=True)
                    nc.scalar.activation(out=t2stage[:, 0:HF + H], in_=ps_t2[:, 0:HF + H],
                                         func=AF.Copy)
                    nc.vector.tensor_copy(out=adst2_s[:, b, :], in_=ps_t2[:, HF + H:W2AUG])
                    xpa = p2.tile([128, F], F16, tag="xpa")
                    nc.vector.tensor_tensor(out=xpa[:], in0=ps_xp[:],
                                            in1=cgcn_s[:], op=ALU.add)
                    nc.vector.tensor_scalar(out=t2stage[:, HF + H:HF + H + F],
                                            in0=xpa[:], scalar1=dinv_s[:, b:b + 1],
                                            scalar2=None, op0=ALU.mult)
                    nc.sync.dma_start(out=comb_shard[128 * b:128 * (b + 1), :],
                                      in_=t2stage[:])
                nc.gpsimd.collective_compute(
                    "AllGather", ALU.bypass, replica_groups=RG,
                    ins=[comb_shard[:]], outs=[comb_full[:]])

              with tc.tile_pool(name="p2sc", bufs=1, space="PSUM") as p2s, \
                   tc.tile_pool(name="p2tc", bufs=2, space="PSUM") as p2t:
                # conv branch (runs during the collective)
                twin_s = p2.tile([32, GPC, 608], F16, tag="twin", bufs=1)
                nc.sync.dma_start(out=twin_s[:], in_=twin_d[:])
                ps_ya = p2s.tile([GPC, 512], F32, space="PSUM", tag="psya")
                ps_yb = p2s.tile([GPC, 96], F32, space="PSUM", tag="psyb")
                for g in range(GPC):
                    nc.tensor.matmul(out=ps_ya[:], lhsT=wsel_s[:, g, :],
                                     rhs=twin_s[:, g, 0:512], start=(g == 0), stop=(g == GPC - 1))
                    nc.tensor.matmul(out=ps_yb[:], lhsT=wsel_s[:, g, :],
                                     rhs=twin_s[:, g, 512:608], start=(g == 0), stop=(g == GPC - 1))
                y_s = p2.tile([GPC, 608], F16, tag="ys")
                nc.vector.tensor_scalar(out=y_s[:, 0:512], in0=ps_ya[:],
                                        scalar1=bconv_s[:], scalar2=0.0,
                                        op0=ALU.add, op1=ALU.max)
                nc.vector.tensor_scalar(out=y_s[:, 512:608], in0=ps_yb[:],
                                        scalar1=bconv_s[:], scalar2=0.0,
                                        op0=ALU.add, op1=ALU.max)
                yt_s = pp.tile([128, 5, GPC], F16, tag="yt")
                nc.vector.memset(yt_s[:], 0.0)
                for i in range(5):
                    c0, c1 = 128 * i, min(128 * (i + 1), 608)
                    psT = p2t.tile([128, 128], F16, space="PSUM", tag="psT")
                    nc.tensor.transpose(out=psT[:c1 - c0, :GPC], in_=y_s[:, c0:c1],
                                        identity=ident_s[:GPC, :GPC])
                    nc.vector.tensor_copy(out=yt_s[0:c1 - c0, i, :], in_=psT[:c1 - c0, :GPC])
                ps_xt = p2s.tile([GPC, 256], F32, space="PSUM", tag="psxt")
                for i in range(5):
                    nc.tensor.matmul(out=ps_xt[:], lhsT=yt_s[:, i, :], rhs=wxt_s[:, i, :],
                                     start=(i == 0), stop=(i == 4))
                xt_s = p2.tile([GPC, 256], F16, tag="xts")
                nc.vector.tensor_tensor(out=xt_s[:], in0=ps_xt[:],
                                        in1=bias_s['bxt'][:],
                                        op=ALU.add)
                xtT_s = pp.tile([128, 2, GPC], F16, tag="xtT")
                for i in range(2):
                    psT = p2t.tile([128, 128], F16, space="PSUM", tag="psT")
                    nc.tensor.transpose(out=psT[:, :GPC], in_=xt_s[:, 128 * i:128 * (i + 1)],
                                        identity=ident_s[:GPC, :GPC])
                    nc.vector.tensor_copy(out=xtT_s[:, i, :], in_=psT[:, :GPC])

            def pool_half(pool, h, nrow=NPC):
                slab = pool.tile([128, 7, CH * PW], F16, tag="slab", bufs=1)
                nc.gpsimd.dma_gather(
                    out_ap=slab[:], in_ap=x1f_dram[0:nrow],
                    idxs_ap=ixp_s[:, h * (CH * PW // 16):(h + 1) * (CH * PW // 16)],
                    num_idxs=CH * PW, num_idxs_reg=CH * PW, elem_size=CROW,
                    transpose=True, single_packet=False)
                for j in range(7):
                    np_ = 128 if j < 6 else HF - 768   # skip NaN pad rows
                    nc.vector.tensor_reduce(
                        out=gmax1T[0:np_, j, h * CH:(h + 1) * CH].rearrange(
                            "p (g one) -> p g one", one=1),
                        in_=slab[0:np_, j, :].rearrange("p (g w) -> p g w", w=PW),
                        op=ALU.max, axis=AX)
                slab2 = pool.tile([128, 1, CH * PW], F16, tag="slab2", bufs=1)
                nc.gpsimd.dma_gather(
                    out_ap=slab2[:], in_ap=x2f_dram[0:nrow],
                    idxs_ap=ixp_s[:, h * (CH * PW // 16):(h + 1) * (CH * PW // 16)],
                    num_idxs=CH * PW, num_idxs_reg=CH * PW, elem_size=XROW,
                    transpose=True, single_packet=False)
                nc.vector.tensor_reduce(
                    out=gmax2T[0:F, 0, h * CH:(h + 1) * CH].rearrange(
                        "p (g one) -> p g one", one=1),
                    in_=slab2[0:F, 0, :].rearrange("p (g w) -> p g w", w=PW),
                    op=ALU.max, axis=AX)

            # ---------------- phase 3: GAT2 + GCN2 edge loop
            with tc.tile_pool(name="p3", bufs=3) as p3, \
                 tc.tile_pool(name="p3g", bufs=2) as p3g, \
                 tc.tile_pool(name="p3s", bufs=3, space="PSUM") as p3s, \
                 tc.tile_pool(name="p3acc", bufs=2, space="PSUM") as p3acc:
                for b in range(NBLK):
                    nt = TPBb[b]
                    ne = 128 * nt
                    v2g = p3g.tile([128, TPB, CROW], F16, tag="v2g")
                    nc.gpsimd.dma_gather(
                        out_ap=v2g[:, 0:nt, :], in_ap=comb_full[:],
                        idxs_ap=ixt2_s[:, b * (EPB // 16):b * (EPB // 16) + ne // 16],
                        num_idxs=ne, num_idxs_reg=ne, elem_size=CROW,
                        single_packet=False)
                    s01_b = p3g.tile([128, EPB], F16, tag="s01b3")
                    nc.sync.dma_start(out=s01_b[:], in_=s01_d[b])
                    s01t_b = p3g.tile([128, EPB], F16, tag="s01tb3")
                    nc.sync.dma_start(out=s01t_b[:], in_=s01t_d[b])
                    ps_agg = p3acc.tile([128, AGGW], F32, space="PSUM",
                                        tag="psagg3", name="psagg3")[:]
                    for k in range(nt):
                        s01_t = s01_b[:, 128 * k:128 * (k + 1)]
                        ps_l = p3s.tile([128, H], F32, space="PSUM", tag="psl3")
                        nc.tensor.matmul(out=ps_l[:], lhsT=s01t_b[:, 128 * k:128 * (k + 1)],
                                         rhs=adst2_s[:, b, :], start=True, stop=False)
                        nc.tensor.matmul(out=ps_l[:], lhsT=ident_s[:],
                                         rhs=v2g[:, k, HF:HF + H], start=False, stop=True)
                        e1 = p3.tile([128, H], F16, tag="e13")
                        nc.scalar.activation(out=e1[:], in_=ps_l[:], func=AF.Exp)
                        e2 = p3.tile([128, H], F16, tag="e23")
                        nc.scalar.activation(out=e2[:], in_=ps_l[:], func=AF.Exp, scale=0.2)
                        ex_dup = p3.tile([128, H, 2], F16, tag="exdup3")
                        exv = p3.tile([128, AGGW], F16, tag="exv3")
                        nc.vector.tensor_tensor(out=exv[:, HF + F:AGGW], in0=e1[:],
                                                in1=e2[:], op=ALU.max)
                        nc.vector.tensor_copy(
                            out=ex_dup[:],
                            in_=exv[:, HF + F:AGGW].rearrange(
                                "p (h one) -> p h one", one=1).to_broadcast([128, H, 2]))
                        nc.vector.tensor_tensor(
                            out=exv[:, 0:HF].rearrange("p (h f2 two) -> p h f2 two",
                                                       h=H, two=2),
                            in0=v2g[:, k, 0:HF].rearrange("p (h f2 two) -> p h f2 two",
                                                          h=H, two=2),
                            in1=ex_dup[:, :, None, :].to_broadcast([128, H, F // 2, 2]),
                            op=ALU.mult)
                        nc.tensor.matmul(out=ps_agg[:, 0:512], lhsT=s01_t,
                                         rhs=exv[:, 0:512], start=(k == 0), stop=(k == nt - 1))
                        nc.tensor.matmul(out=ps_agg[:, 512:HF], lhsT=s01_t,
                                         rhs=exv[:, 512:HF], start=(k == 0), stop=False)
                        nc.tensor.matmul(out=ps_agg[:, HF:HF + F], lhsT=s01_t,
                                         rhs=v2g[:, k, HF + H:HF + H + F],
                                         start=False, stop=False)
                        nc.tensor.matmul(out=ps_agg[:, HF + F:AGGW], lhsT=s01_t,
                                         rhs=exv[:, HF + F:AGGW], start=False, stop=(k == nt - 1))
                    rec = p3.tile([128, H], F32, tag="rec3")
                    nc.vector.reciprocal(out=rec[:], in_=ps_agg[:, HF + F:AGGW])
                    rec_dup = p3.tile([128, H, 2], F16, tag="recdup3")
                    nc.vector.tensor_copy(out=rec_dup[:],
                                          in_=rec[:, :, None].to_broadcast([128, H, 2]))
                    aggf = p3.tile([128, HF], F16, tag="aggf3")
                    nc.scalar.activation(out=aggf[:], in_=ps_agg[:, 0:HF], func=AF.Copy)
                    u_s = p3.tile([128, HF], F16, tag="us")
                    nc.vector.tensor_tensor(
                        out=u_s[:].rearrange("p (h f2 two) -> p h f2 two", h=H, two=2),
                        in0=aggf[:].rearrange("p (h f2 two) -> p h f2 two",
                                              h=H, two=2),
                        in1=rec_dup[:, :, None, :].to_broadcast([128, H, F // 2, 2]),
                        op=ALU.mult)
                    # x1f = relu(u + bg2) -> persistent SBUF + DRAM (for pool gather)
                    v_s = p3.tile([128, HF], F16, tag="vs")
                    nc.vector.tensor_tensor(out=v_s[:], in0=u_s[:], in1=bg2_s[:], op=ALU.add)
                    nc.scalar.activation(out=x1f_s[:, b, :], in_=v_s[:], func=AF.Relu)
                    nc.sync.dma_start(out=x1f_dram[128 * b:128 * (b + 1), 0:HF],
                                      in_=x1f_s[:, b, :])
                    # x2f = relu(dinv*gcn_agg + bgcn)
                    g2s = p3.tile([128, F], F32, tag="g2s")
                    nc.scalar.activation(out=g2s[:], in_=ps_agg[:, HF:HF + F],
                                         func=AF.Copy, scale=dinv_s[:, b:b + 1])
                    g2f = p3.tile([128, F], F32, tag="g2f")
                    nc.vector.tensor_tensor(out=g2f[:], in0=g2s[:],
                                            in1=bgcnr_s[:], op=ALU.add)
                    nc.scalar.activation(out=x2f_s[:, b, :], in_=g2f[:], func=AF.Relu)
                    nc.sync.dma_start(out=x2f_dram[128 * b:128 * (b + 1), 0:F],
                                      in_=x2f_s[:, b, :])
                    if b == BH - 1:
                        pool_half(p3g, 0, nrow=128 * BH)

            # ---------------- phase 4: pooling + head
            with tc.tile_pool(name="p4", bufs=2) as p4:
              with tc.tile_pool(name="p4s", bufs=1, space="PSUM") as p4s:
                pool_half(p4, 1)
              # means via matmul, then transpose
              with tc.tile_pool(name="p4sm", bufs=1, space="PSUM") as p4s:
                ps_m1 = p4s.tile([GPC, HF], F32, space="PSUM", tag="psm1")
                ps_m2 = p4s.tile([GPC, F], F32, space="PSUM", tag="psm2")
                for b in range(NBLK):
                    nc.tensor.matmul(out=ps_m1[:, 0:512], lhsT=mmean_s[:, b, :],
                                     rhs=x1f_s[:, b, 0:512], start=(b == 0), stop=(b == NBLK - 1))
                    nc.tensor.matmul(out=ps_m1[:, 512:HF], lhsT=mmean_s[:, b, :],
                                     rhs=x1f_s[:, b, 512:HF], start=(b == 0), stop=(b == NBLK - 1))
                    nc.tensor.matmul(out=ps_m2[:], lhsT=mmean_s[:, b, :],
                                     rhs=x2f_s[:, b, :], start=(b == 0), stop=(b == NBLK - 1))
                mean1 = p4.tile([GPC, HF], F16, tag="mean1")
                nc.vector.tensor_copy(out=mean1[:], in_=ps_m1[:])
                mean2 = p4.tile([GPC, F], F16, tag="mean2")
                nc.vector.tensor_copy(out=mean2[:], in_=ps_m2[:])
              with tc.tile_pool(name="p4sh", bufs=1, space="PSUM") as p4s:
                gmean1T = pp.tile([128, 7, GPC], F16, tag="gmean1T")
                nc.vector.memset(gmean1T[:], 0.0)
                gmean2T = pp.tile([128, 1, GPC], F16, tag="gmean2T")
                nc.vector.memset(gmean2T[:], 0.0)
                for i in range(7):
                    c0, c1 = 128 * i, min(128 * (i + 1), HF)
                    psT = p4s.tile([128, 128], F16, space="PSUM", tag="psT4", bufs=2)
                    nc.tensor.transpose(out=psT[:c1 - c0, :GPC], in_=mean1[:, c0:c1],
                                        identity=ident_s[:GPC, :GPC])
                    nc.vector.tensor_copy(out=gmean1T[0:c1 - c0, i, :], in_=psT[:c1 - c0, :GPC])
                psT = p4s.tile([128, 128], F16, space="PSUM", tag="psT4", bufs=2)
                nc.tensor.transpose(out=psT[:F, :GPC], in_=mean2[:], identity=ident_s[:GPC, :GPC])
                nc.vector.tensor_copy(out=gmean2T[0:F, 0, :], in_=psT[:F, :GPC])

                def head_mm(ps, chunks, rhs_tile, nw):
                    n = len(chunks)
                    for i, ch in enumerate(chunks):
                        nc.tensor.matmul(out=ps[:], lhsT=ch, rhs=rhs_tile[:, i, :nw],
                                         start=(i == 0), stop=(i == n - 1))

                def bias_relu_T(ps, bias_ap, w, relu, nT, tagb):
                    zs = p4.tile([GPC, w], F16, tag="z" + tagb)
                    nc.vector.tensor_tensor(out=zs[:], in0=ps[:],
                                            in1=bias_ap, op=ALU.add)
                    if relu:
                        nc.vector.tensor_scalar(out=zs[:], in0=zs[:], scalar1=0.0,
                                                scalar2=None, op0=ALU.max)
                    zT = pp.tile([128, nT, GPC], F16, tag="zT" + tagb)
                    for i in range(nT):
                        psT2 = p4s.tile([128, 128], F16, space="PSUM", tag="psT4", bufs=2)
                        nc.tensor.transpose(out=psT2[:, :GPC], in_=zs[:, 128 * i:128 * (i + 1)],
                                            identity=ident_s[:GPC, :GPC])
                        nc.vector.tensor_copy(out=zT[:, i, :], in_=psT2[:, :GPC])
                    return zT

                ps_z1 = p4s.tile([GPC, 128], F32, space="PSUM", tag="psz1")
                head_mm(ps_z1, [gmax1T[:, j, :] for j in range(7)]
                        + [gmean1T[:, j, :] for j in range(7)], wfg1_s, 128)
                z1T = bias_relu_T(ps_z1, bias_s['bfg1'][:], 128, True, 1, "1")
                ps_z2 = p4s.tile([GPC, 128], F32, space="PSUM", tag="psz2")
                head_mm(ps_z2, [gmax2T[:, 0, :], gmean2T[:, 0, :]], wfg2_s, 128)
                z2T = bias_relu_T(ps_z2, bias_s['bfg2'][:], 128, True, 1, "2")
                ps_h1 = p4s.tile([GPC, 512], F32, space="PSUM", tag="psh1")
                head_mm(ps_h1, [z1T[:, 0, :], z2T[:, 0, :], xtT_s[:, 0, :], xtT_s[:, 1, :]],
                        w1_s, 512)
                h1T = bias_relu_T(ps_h1, bias_s['b1'][:], 512, True, 4, "h1")
                ps_h2 = p4s.tile([GPC, 256], F32, space="PSUM", tag="psh2")
                head_mm(ps_h2, [h1T[:, i, :] for i in range(4)], w2_s, 256)
                h2T = bias_relu_T(ps_h2, bias_s['b2'][:], 256, True, 2, "h2")
                ps_o = p4s.tile([GPC, 1], F32, space="PSUM", tag="pso")
                head_mm(ps_o, [h2T[:, i, :] for i in range(2)], wo_s, 1)
                o_s = p4.tile([GPC, 1], F32, tag="os")
                nc.vector.tensor_scalar(out=o_s[:], in0=ps_o[:], scalar1=bo_s[:],
                                        scalar2=None, op0=ALU.add)
                nc.sync.dma_start(out=out_d[:], in_=o_s[:])

    nc.compile()
    return nc


def build_in_maps(nc, shared, cores):
    declared = set()
    import concourse.mybir as _mb
    for alloc in nc.m.functions[0].allocations:
        if isinstance(alloc, _mb.MemoryLocationSet) and alloc.kind == "ExternalInput":
            declared.add(alloc.memorylocations[0].name)
    in_maps = []
    for c in range(8):
        m = dict(shared)
        m.update(cores[c])
        in_maps.append({k: np.ascontiguousarray(v) for k, v in m.items()
                        if k in declared})
    return in_maps


_CACHE = {}


def run_device(inputs):
    meta, shared, cores = prep(**inputs)
    key = (meta['NBLK'], meta['TPB'], meta['PW'], meta['TPBb'], meta['BH'])
    if key not in _CACHE:
        _CACHE[key] = build(meta)
    nc = _CACHE[key]
    in_maps = build_in_maps(nc, shared, cores)
    res = run_bass_kernel_spmd(nc, in_maps, core_ids=list(range(8)))
    out = np.concatenate([res.results[c]['out'] for c in range(8)], axis=0)
    return out.astype(np.float32)


def kernel(**inputs):
    return run_device(inputs)
